# revision 2
# baseline (speedup 1.0000x reference)
"""KNRM kernel for 8 Trainium2 NeuronCores (data-parallel over batch).

Pipeline per core (32 batches):
  - host: augment embed table with precomputed 1/||row|| column; reorder token
    index tensors into the SBUF layouts the device program expects
  - device: indirect-DMA gather of embedding rows (the memory-bound core of
    the problem), row-normalize on DVE, PE transposes into [e, tok] layout,
    fp32r cosine matmuls (4 batches packed per PSUM bank via col tiling),
    Gaussian-kernel pooling on ACT (Square + Exp with free-dim accumulation),
    log/mask/selection-matmul tail, dense head.
Masking is folded into the contraction: an extra "bias" row appended to the
e-dimension drives masked doc positions to cosine=-1e6 (=> all kernels
underflow to exactly 0), and masked query rows are zeroed via the
normalization scale so the final q_mask multiply kills them.
"""

import sys

sys.path.insert(0, "/opt/trn_rl_repo")

import numpy as np

B, Q, D, V, E = 256, 20, 512, 100000, 300
NCORES = 8
BLOC = B // NCORES  # 32 batches per core
SLOT = 304  # 300 emb + 1 rs + 3 pad
QPAD = 32  # query slots per batch (20 real + 12 pad)
QSLOTS = BLOC * QPAD // 128  # 8 -> q idx tile [128, 8]
DCHUNKS = 8  # doc chunks per core
DCTOK = 2048  # doc tokens per chunk (= 4 batches)
DSLOT = DCTOK // 128  # 16 slots per chunk
NK = 11

MASK_BIAS = -1.0e6


def _mus(n):
    l = [1.0]
    bs = 2.0 / (n - 1)
    l.append(1 - bs / 2)
    for i in range(1, n - 1):
        l.append(l[i] - bs)
    return l


def _sigmas(n):
    bs = 2.0 / (n - 1)
    return [0.0001] + [0.5 * bs] * (n - 1)


MUS = _mus(NK)
GS = [1.0 / (2.0 * s * s) for s in _sigmas(NK)]  # 5e7, 50, 50, ...

_prog_cache = {}
DEBUG = False


def _build_program():
    key = ("nc", DEBUG)
    if key in _prog_cache:
        return _prog_cache[key]

    import concourse.bass as bass
    import concourse.bacc as bacc
    import concourse.mybir as mybir
    import concourse.tile as tile

    f32 = mybir.dt.float32
    f32r = mybir.dt.float32r
    bf16 = mybir.dt.bfloat16
    i32 = mybir.dt.int32
    AF = mybir.ActivationFunctionType
    ALU = mybir.AluOpType

    nc = bacc.Bacc(
        "TRN2", target_bir_lowering=False, debug=False, num_devices=NCORES
    )

    table = nc.dram_tensor("table", [V, SLOT], f32, kind="ExternalInput").ap()
    d_idx = nc.dram_tensor(
        "d_idx", [DCHUNKS, 128, DSLOT], i32, kind="ExternalInput"
    ).ap()
    q_idx = nc.dram_tensor("q_idx", [128, QSLOTS], i32, kind="ExternalInput").ap()
    ident = nc.dram_tensor("ident", [128, 128], f32, kind="ExternalInput").ap()
    s_sel = nc.dram_tensor("s_sel", [128, 4], f32, kind="ExternalInput").ap()
    s_selT = nc.dram_tensor("s_selT", [4, 128], f32, kind="ExternalInput").ap()
    d_tokf = nc.dram_tensor(
        "d_tokf", [DCHUNKS, 4, 512], f32, kind="ExternalInput"
    ).ap()
    w4 = nc.dram_tensor("w4", [4, NK], f32, kind="ExternalInput").ap()
    negmu = nc.dram_tensor("negmu", [128, NK], f32, kind="ExternalInput").ap()
    b4 = nc.dram_tensor("b4", [4, 1], f32, kind="ExternalInput").ap()
    out = nc.dram_tensor("out", [4, DCHUNKS], f32, kind="ExternalOutput").ap()
    dbg_pkq = (
        nc.dram_tensor("dbg_pkq", [DCHUNKS, 128, NK], f32, kind="ExternalOutput").ap()
        if DEBUG
        else None
    )
    dbg_cos = (
        nc.dram_tensor("dbg_cos", [DCHUNKS, 128, 512], f32, kind="ExternalOutput").ap()
        if DEBUG
        else None
    )
    dbg_de = (
        nc.dram_tensor("dbg_de", [128, DSLOT * SLOT], f32, kind="ExternalOutput").ap()
        if DEBUG
        else None
    )

    with tile.TileContext(nc) as tc:
        import contextlib

        with contextlib.ExitStack() as ctx:
            const_pool = ctx.enter_context(tc.tile_pool(name="consts", bufs=1))
            qp = ctx.enter_context(tc.tile_pool(name="qprep", bufs=1))
            dpool = ctx.enter_context(tc.tile_pool(name="demb", bufs=2))
            dtpool = ctx.enter_context(tc.tile_pool(name="dT", bufs=2))
            sqpool = ctx.enter_context(tc.tile_pool(name="sq", bufs=2))
            pkpool = ctx.enter_context(tc.tile_pool(name="pk", bufs=1))
            psum = ctx.enter_context(
                tc.tile_pool(name="psum", bufs=2, space="PSUM")
            )

            ident_t = const_pool.tile([128, 128], f32)
            nc.sync.dma_start(out=ident_t[:], in_=ident[:])
            s_sel_t = const_pool.tile([128, 4], f32)
            nc.sync.dma_start(out=s_sel_t[:], in_=s_sel[:])
            s_selT_t = const_pool.tile([4, 128], f32)
            nc.sync.dma_start(out=s_selT_t[:], in_=s_selT[:])
            w4_t = const_pool.tile([4, NK], f32)
            nc.sync.dma_start(out=w4_t[:], in_=w4[:])
            b4_t = const_pool.tile([4, 1], f32)
            nc.sync.dma_start(out=b4_t[:], in_=b4[:])
            negmu_t = const_pool.tile([128, NK], f32)
            nc.sync.dma_start(out=negmu_t[:], in_=negmu[:])

            # ---------------- Q preparation ----------------
            qi = qp.tile([128, QSLOTS], i32)
            nc.sync.dma_start(out=qi[:], in_=q_idx[:])

            qe = qp.tile([128, QSLOTS * SLOT], f32)
            qe3 = qe[:].rearrange("p (s c) -> p s c", c=SLOT)
            for s in range(QSLOTS):
                nc.gpsimd.indirect_dma_start(
                    out=qe3[:, s, :],
                    out_offset=None,
                    in_=table[:],
                    in_offset=bass.IndirectOffsetOnAxis(ap=qi[:, s : s + 1], axis=0),
                )

            # query mask (tok > 0) and masked rs column
            qm = qp.tile([128, QSLOTS], f32)
            nc.vector.tensor_scalar(
                out=qm[:], in0=qi[:], scalar1=0, scalar2=None, op0=ALU.is_gt
            )
            rsm = qp.tile([128, QSLOTS], f32)
            nc.vector.tensor_tensor(
                out=rsm[:], in0=qm[:], in1=qe3[:, :, 300:301], op=ALU.mult
            )
            qtokf = qp.tile([128, QSLOTS], f32)
            nc.vector.tensor_copy(out=qtokf[:], in_=qi[:])
            # 0.01 * q_mask for the log tail
            qm001 = qp.tile([128, QSLOTS], f32)
            nc.vector.tensor_scalar(
                out=qm001[:], in0=qm[:], scalar1=0.01, scalar2=None, op0=ALU.mult
            )

            # normalize+mask query rows; set the appended-one column
            for s in range(QSLOTS):
                nc.vector.tensor_scalar(
                    out=qe3[:, s, 0:300],
                    in0=qe3[:, s, 0:300],
                    scalar1=rsm[:, s : s + 1],
                    scalar2=None,
                    op0=ALU.mult,
                )
            nc.scalar.activation(
                out=qe3[:, :, 300:301],
                in_=qe3[:, :, 300:301],
                func=AF.Identity,
                bias=1.0,
                scale=0.0,
            )

            # transpose q into [e, tok] slabs: qT0/qT1 [128, 1024], qT2 [48, 1024]
            qT = [
                qp.tile([128, 128 * QSLOTS], bf16, tag=f"qT{c}", name=f"qT{c}")
                for c in range(3)
            ]
            for j in range(QSLOTS):
                pt = psum.tile([128, 1536], f32, tag="dT", name="qtp")
                nc.tensor.transpose(
                    out=pt[:, 0:128], in_=qe3[:, j, 0:128], identity=ident_t[:]
                )
                nc.tensor.transpose(
                    out=pt[:, 128:256],
                    in_=qe3[:, j, 128:256],
                    identity=ident_t[:],
                )
                nc.tensor.transpose(
                    out=pt[0:48, 256:384],
                    in_=qe3[:, j, 256:304],
                    identity=ident_t[:],
                )
                nc.vector.tensor_copy(
                    out=qT[0][:, j * 128 : (j + 1) * 128], in_=pt[:, 0:128]
                )
                nc.vector.tensor_copy(
                    out=qT[1][:, j * 128 : (j + 1) * 128], in_=pt[:, 128:256]
                )
                nc.vector.tensor_copy(
                    out=qT[2][0:45, j * 128 : (j + 1) * 128],
                    in_=pt[0:45, 256:384],
                )

            # ---------------- main loop over doc chunks ----------------
            pkq_tiles = []
            for h in range(DCHUNKS):
                di = dpool.tile([128, DSLOT], i32, tag="didx")
                nc.sync.dma_start(out=di[:], in_=d_idx[h])

                de = dpool.tile([128, DSLOT * SLOT], f32, tag="demb")
                de3 = de[:].rearrange("p (s c) -> p s c", c=SLOT)
                for s in range(DSLOT):
                    nc.gpsimd.indirect_dma_start(
                        out=de3[:, s, :],
                        out_offset=None,
                        in_=table[:],
                        in_offset=bass.IndirectOffsetOnAxis(ap=di[:, s : s + 1], axis=0),
                    )

                # normalize rows (no mask folded here)
                for s in range(DSLOT):
                    nc.vector.tensor_scalar(
                        out=de3[:, s, 0:300],
                        in0=de3[:, s, 0:300],
                        scalar1=de3[:, s, 300:301],
                        scalar2=None,
                        op0=ALU.mult,
                    )
                # doc mask bias column: 0 for valid, -1e6 for masked
                dm = dpool.tile([128, DSLOT], f32, tag="dmask")
                nc.vector.tensor_scalar(
                    out=dm[:], in0=di[:], scalar1=0, scalar2=None, op0=ALU.is_gt
                )
                nc.vector.tensor_scalar(
                    out=de3[:, :, 300:301],
                    in0=dm[:],
                    scalar1=-MASK_BIAS,
                    scalar2=MASK_BIAS,
                    op0=ALU.mult,
                    op1=ALU.add,
                )

                dtf = dpool.tile([4, 512], f32, tag="dtokf")
                nc.sync.dma_start(out=dtf[:], in_=d_tokf[h])

                if DEBUG and h == 0:
                    nc.sync.dma_start(out=dbg_de[:], in_=de[:])

                cos = psum.tile([128, 512], f32, tag="cos")
                for beta in range(4):
                    # transposes for batch beta (tiles j = 4*beta .. 4*beta+4)
                    pt = psum.tile([128, 1536], f32, tag="dT")
                    for t in range(4):
                        j = 4 * beta + t
                        nc.tensor.transpose(
                            out=pt[:, t * 128 : (t + 1) * 128],
                            in_=de3[:, j, 0:128],
                            identity=ident_t[:],
                        )
                        nc.tensor.transpose(
                            out=pt[:, 512 + t * 128 : 512 + (t + 1) * 128],
                            in_=de3[:, j, 128:256],
                            identity=ident_t[:],
                        )
                        nc.tensor.transpose(
                            out=pt[0:48, 1024 + t * 128 : 1024 + (t + 1) * 128],
                            in_=de3[:, j, 256:304],
                            identity=ident_t[:],
                        )
                    dT0 = dtpool.tile([128, 512], bf16, tag="dT0")
                    dT1 = dtpool.tile([128, 512], bf16, tag="dT1")
                    dT2 = dtpool.tile([48, 512], bf16, tag="dT2")
                    nc.scalar.copy(out=dT0[:], in_=pt[:, 0:512])
                    nc.vector.tensor_copy(out=dT1[:], in_=pt[:, 512:1024])
                    nc.scalar.copy(out=dT2[0:45, :], in_=pt[0:45, 1024:1536])

                    b_glob = 4 * h + beta
                    qs = QPAD * b_glob
                    for c in range(3):
                        if c < 2:
                            lhs = qT[c][:, qs : qs + QPAD]
                            rhs = (dT0 if c == 0 else dT1)[:]
                        else:
                            lhs = qT[2][0:45, qs : qs + QPAD]
                            rhs = dT2[0:45, :]
                        nc.tensor.matmul(
                            out=cos[32 * beta : 32 * beta + 32, :],
                            lhsT=lhs,
                            rhs=rhs,
                            start=(c == 0),
                            stop=(c == 2),
                            tile_position=(0, 32 * beta),
                        )

                # k0 (sigma=1e-4) = exact-token-match count: broadcast doc
                # token rows to all partitions via a tiny PE outer product,
                # then fused is_equal + free-dim accumulate on DVE
                pkq = pkpool.tile([128, NK], f32, tag=f"pkq{h}")
                pkq_tiles.append(pkq)
                ptb = psum.tile([128, 1536], f32, tag="dT", name="ptb")
                nc.tensor.matmul(
                    out=ptb[:, 0:512],
                    lhsT=s_selT_t[:],
                    rhs=dtf[:],
                    start=True,
                    stop=True,
                )
                cmp = sqpool.tile([128, 512], f32, tag="cmp")
                nc.vector.tensor_scalar(
                    out=cmp[:],
                    in0=ptb[:, 0:512],
                    scalar1=qtokf[:, h : h + 1],
                    scalar2=0.0,
                    op0=ALU.is_equal,
                    op1=ALU.add,
                    accum_out=pkq[:, 0:1],
                )

                if DEBUG:
                    cos_sb = sqpool.tile([128, 512], f32, tag="cossb", name="cos_sb")
                    nc.vector.tensor_copy(out=cos_sb[:], in_=cos[:])
                    nc.sync.dma_start(out=dbg_cos[h], in_=cos_sb[:])
                # Gaussian kernel pooling k=1..10:
                # pkq[:, k] = sum_d exp(-g_k (c-mu_k)^2)
                sq = sqpool.tile([128, 512], f32, tag="sq")
                scr = sqpool.tile([128, 512], f32, tag="scr")
                for k in range(1, NK):
                    nc.scalar.activation(
                        out=sq[:],
                        in_=cos[:],
                        func=AF.Square,
                        bias=negmu_t[:, k : k + 1],
                    )
                    nc.scalar.activation(
                        out=scr[:],
                        in_=sq[:],
                        func=AF.Exp,
                        scale=-GS[k],
                        accum_out=pkq[:, k : k + 1],
                    )

            if DEBUG:
                for h in range(DCHUNKS):
                    nc.sync.dma_start(out=dbg_pkq[h], in_=pkq_tiles[h][:])
            # ---------------- tail: log, mask, per-batch reduce, dense ----------------
            out_acc = pkpool.tile([4, DCHUNKS], f32, tag="outacc")
            for h in range(DCHUNKS):
                pkq = pkq_tiles[h]
                nc.vector.tensor_scalar(
                    out=pkq[:], in0=pkq[:], scalar1=1e-10, scalar2=None, op0=ALU.max
                )
                lnp = pkpool.tile([128, NK], f32, tag=f"lnp{h}")
                nc.scalar.activation(out=lnp[:], in_=pkq[:], func=AF.Ln)
                nc.vector.tensor_scalar(
                    out=lnp[:],
                    in0=lnp[:],
                    scalar1=qm001[:, h : h + 1],
                    scalar2=None,
                    op0=ALU.mult,
                )
                pkp = psum.tile([4, NK], f32, tag="cos")
                nc.tensor.matmul(
                    out=pkp[:],
                    lhsT=s_sel_t[:],
                    rhs=lnp[:],
                    start=True,
                    stop=True,
                )
                pks = pkpool.tile([4, NK], f32, tag=f"pks{h}")
                nc.vector.tensor_tensor(
                    out=pks[:], in0=pkp[:], in1=w4_t[:], op=ALU.mult
                )
                nc.vector.reduce_sum(
                    out=out_acc[:, h : h + 1], in_=pks[:], axis=mybir.AxisListType.X
                )
            nc.scalar.activation(
                out=out_acc[:],
                in_=out_acc[:],
                func=AF.Identity,
                bias=b4_t[:, 0:1],
                scale=1.0,
            )
            nc.sync.dma_start(out=out[:], in_=out_acc[:])

    nc.compile()
    _prog_cache[key] = nc
    return nc


def _host_prep(query_tokens, doc_tokens, embed_table, dense_w, dense_b):
    emb = np.ascontiguousarray(embed_table, dtype=np.float32)
    norms = np.sqrt(np.sum(emb.astype(np.float64) ** 2, axis=1))
    rs = (1.0 / np.maximum(norms, 1e-13)).astype(np.float32)
    table = np.zeros((V, SLOT), dtype=np.float32)
    table[:, :E] = emb
    table[:, E] = rs

    qt = np.asarray(query_tokens).astype(np.int32)
    dt = np.asarray(doc_tokens).astype(np.int32)

    in_maps = []
    for c in range(NCORES):
        dt_c = dt[c * BLOC : (c + 1) * BLOC].reshape(-1)  # [16384]
        # chunk h, slot j, partition p <- token 2048h + 128j + p
        d_idx = np.ascontiguousarray(
            dt_c.reshape(DCHUNKS, DSLOT, 128).transpose(0, 2, 1)
        )

        qt_c = qt[c * BLOC : (c + 1) * BLOC]  # [32, 20]
        q_pad = np.zeros((BLOC, QPAD), dtype=np.int32)
        q_pad[:, :Q] = qt_c
        qf = q_pad.reshape(-1)  # [1024], slot s = 32b + i
        q_idx = np.ascontiguousarray(qf.reshape(QSLOTS, 128).T)

        s_sel = np.zeros((128, 4), dtype=np.float32)
        for p in range(128):
            s_sel[p, p // 32] = 1.0

        # doc tokens as f32 rows [chunk, batch-in-chunk, 512] for the k0 path
        d_tokf = (
            dt[c * BLOC : (c + 1) * BLOC]
            .reshape(DCHUNKS, 4, 512)
            .astype(np.float32)
        )

        in_maps.append(
            {
                "table": table,
                "d_idx": d_idx,
                "q_idx": q_idx,
                "ident": np.eye(128, dtype=np.float32),
                "s_sel": s_sel,
                "s_selT": np.ascontiguousarray(s_sel.T),
                "d_tokf": d_tokf,
                "w4": np.tile(
                    np.asarray(dense_w, dtype=np.float32).reshape(1, NK), (4, 1)
                ),
                "b4": np.full((4, 1), np.asarray(dense_b).reshape(-1)[0], np.float32),
                "negmu": np.tile(
                    -np.asarray(MUS, dtype=np.float32).reshape(1, NK), (128, 1)
                ),
            }
        )
    return in_maps


def _install_loud_hook():
    # surface exceptions raised inside the PJRT compile callback, which are
    # otherwise swallowed by the C++ layer
    import traceback
    from concourse import bass2jax

    if getattr(bass2jax, "_loud_hook_installed", False):
        return
    orig = bass2jax.neuronx_cc_hook

    def loud(*a, **k):
        try:
            return orig(*a, **k)
        except BaseException:
            traceback.print_exc()
            raise

    bass2jax.neuronx_cc_hook = loud
    bass2jax._loud_hook_installed = True


LAST_RESULTS = None


def kernel(query_tokens, doc_tokens, embed_table, dense_w, dense_b):
    global LAST_RESULTS
    _install_loud_hook()
    from concourse.bass_utils import run_bass_kernel_spmd

    nc = _build_program()
    in_maps = _host_prep(query_tokens, doc_tokens, embed_table, dense_w, dense_b)
    res = run_bass_kernel_spmd(nc, in_maps, list(range(NCORES)))
    LAST_RESULTS = res
    out = np.empty((B,), dtype=np.float32)
    for c in range(NCORES):
        arr = res.results[c]["out"]  # [4, 8]: batch 4h+beta at [beta, h]
        out[c * BLOC : (c + 1) * BLOC] = arr.T.reshape(BLOC)
    return out



# revision 18
# speedup vs baseline: 1.5105x; 1.5105x over previous
"""KNRM kernel for 8 Trainium2 NeuronCores (data-parallel over batch).

Per core (32 batches):
  - host: dedup this core's tokens (~16k unique < int16 max), build a
    pre-normalized bf16 mini-table [17472, 384] (300 emb dims + mask-bias
    column at 300: -1e6 for vocab id 0, else 0), remap token tensors to
    int16 mini-table indices replicated across 16-partition groups (the
    Q7 dma_gather ucode reads a copy per 16-partition channel group).
  - device: per 2048-token chunk, ONE dma_gather(transpose=True) delivers
    embeddings directly in [e, token] layout (partition p, free slot j
    holds element 128j+p), so the cosine matmuls need no PE transposes and
    no PSUM->SBUF copies. Masking is folded into the contraction via the
    bias column (query side forced to 1.0). Gaussian kernel pooling runs
    as ONE scalar-engine pass per kernel using Derivative_Erf
    (d/dx erf = 2/sqrt(pi) * exp(-x^2)) with free-dim accumulation,
    reading cos straight from PSUM; the 2/sqrt(pi) factor is undone by
    the Ln(scale=sqrt(pi)/2) in the tail. k0 (sigma=1e-4, exact token
    match) is a DVE token-equality count scaled by 2/sqrt(pi) to share
    the same tail.
"""

import sys

sys.path.insert(0, "/opt/trn_rl_repo")

import numpy as np

B, Q, D, V, E = 256, 20, 512, 100000, 300
NCORES = 8
BLOC = B // NCORES  # 32 batches per core
QPAD = 32  # query slots per batch (20 real + 12 pad)
NQTOK = BLOC * QPAD  # 1024 query gather slots per core
DCHUNKS = 8  # doc chunks per core
DCTOK = 2048  # doc tokens per chunk (= 4 batches)
NK = 11
MSLOT = 384  # mini-table row elems (bf16) -> 768B, 256B-multiple
MROWS = 17472  # >= max unique tokens per core (16384 doc + 1024 q)
BIAS_COL = 300
MASK_BIAS = -1.0e6

SQRT50 = float(np.sqrt(50.0))
LN_SCALE = float(np.sqrt(np.pi) / 2.0)  # undo derf's 2/sqrt(pi)
K0_SCALE = float(2.0 / np.sqrt(np.pi))
CLIP = 1e-10 / LN_SCALE


def _mus(n):
    l = [1.0]
    bs = 2.0 / (n - 1)
    l.append(1 - bs / 2)
    for i in range(1, n - 1):
        l.append(l[i] - bs)
    return l


MUS = _mus(NK)

_prog_cache = {}
DEBUG = False


def _build_program():
    key = ("nc", DEBUG)
    if key in _prog_cache:
        return _prog_cache[key]

    import concourse.bass as bass
    import concourse.bacc as bacc
    import concourse.mybir as mybir
    import concourse.tile as tile
    from concourse import library_config

    f32 = mybir.dt.float32
    bf16 = mybir.dt.bfloat16
    i16 = mybir.dt.int16
    AF = mybir.ActivationFunctionType
    ALU = mybir.AluOpType

    nc = bacc.Bacc(
        "TRN2", target_bir_lowering=False, debug=False, num_devices=NCORES
    )

    mtab = nc.dram_tensor("mtab", [MROWS, MSLOT], bf16, kind="ExternalInput").ap()
    d_idx = nc.dram_tensor(
        "d_idx", [DCHUNKS, 4, 128, 512 // 16], i16, kind="ExternalInput"
    ).ap()
    q_idx = nc.dram_tensor(
        "q_idx", [2, 128, 512 // 16], i16, kind="ExternalInput"
    ).ap()
    s_sel = nc.dram_tensor("s_sel", [128, 4], f32, kind="ExternalInput").ap()
    s_selT = nc.dram_tensor("s_selT", [4, 128], f32, kind="ExternalInput").ap()
    d_tokf = nc.dram_tensor(
        "d_tokf", [DCHUNKS, 4, 512], f32, kind="ExternalInput"
    ).ap()
    q_tokf = nc.dram_tensor("q_tokf", [128, DCHUNKS], f32, kind="ExternalInput").ap()
    qm001 = nc.dram_tensor("qm001", [128, DCHUNKS], f32, kind="ExternalInput").ap()
    w4 = nc.dram_tensor("w4", [4, NK], f32, kind="ExternalInput").ap()
    b4 = nc.dram_tensor("b4", [4, 1], f32, kind="ExternalInput").ap()
    derfb = nc.dram_tensor("derfb", [128, NK], f32, kind="ExternalInput").ap()
    qones = nc.dram_tensor("qones", [1, NQTOK], bf16, kind="ExternalInput").ap()
    out = nc.dram_tensor("out", [4, DCHUNKS], f32, kind="ExternalOutput").ap()
    dbg_pkq = (
        nc.dram_tensor("dbg_pkq", [DCHUNKS, 128, NK], f32, kind="ExternalOutput").ap()
        if DEBUG
        else None
    )
    dbg_cos = (
        nc.dram_tensor("dbg_cos", [DCHUNKS, 128, 512], f32, kind="ExternalOutput").ap()
        if DEBUG
        else None
    )

    with tile.TileContext(nc) as tc:
        import contextlib

        with contextlib.ExitStack() as ctx:
            const_pool = ctx.enter_context(tc.tile_pool(name="consts", bufs=1))
            qp = ctx.enter_context(tc.tile_pool(name="qprep", bufs=1))
            dpool = ctx.enter_context(tc.tile_pool(name="demb", bufs=2))
            pkpool = ctx.enter_context(tc.tile_pool(name="pk", bufs=1))
            scr = ctx.enter_context(tc.tile_pool(name="scr", bufs=2))
            psum = ctx.enter_context(
                tc.tile_pool(name="psum", bufs=2, space="PSUM")
            )

            nc.gpsimd.load_library(library_config.mlp)

            s_sel_t = const_pool.tile([128, 4], f32)
            nc.sync.dma_start(out=s_sel_t[:], in_=s_sel[:])
            s_selT_t = const_pool.tile([4, 128], f32)
            nc.sync.dma_start(out=s_selT_t[:], in_=s_selT[:])
            w4_t = const_pool.tile([4, NK], f32)
            nc.sync.dma_start(out=w4_t[:], in_=w4[:])
            b4_t = const_pool.tile([4, 1], f32)
            nc.sync.dma_start(out=b4_t[:], in_=b4[:])
            derfb_t = const_pool.tile([128, NK], f32)
            nc.sync.dma_start(out=derfb_t[:], in_=derfb[:])
            qtokf_t = const_pool.tile([128, DCHUNKS], f32)
            nc.sync.dma_start(out=qtokf_t[:], in_=q_tokf[:])
            qm001_t = const_pool.tile([128, DCHUNKS], f32)
            nc.sync.dma_start(out=qm001_t[:], in_=qm001[:])

            # ---------------- Q gather (transposed), 2x512 idxs ----------------
            qT = qp.tile([128, 2 * 3 * 512], bf16)
            qT4 = qT[:].rearrange("p (g j n) -> p g j n", g=2, j=3)
            for g in range(2):
                qi = qp.tile([128, 512 // 16], i16, tag=f"qi{g}", name=f"qi{g}")
                nc.sync.dma_start(out=qi[:], in_=q_idx[g])
                nc.gpsimd.dma_gather(
                    out_ap=qT4[:, g],
                    in_ap=mtab[:],
                    idxs_ap=qi[:],
                    num_idxs=512,
                    num_idxs_reg=512,
                    elem_size=MSLOT,
                    transpose=True,
                )
                # query-side bias multiplier: force e-row 300 (tile 2, part 44)
                nc.sync.dma_start(
                    out=qT4[44:45, g : g + 1, 2, :], in_=qones[:, 512 * g : 512 * (g + 1)]
                )

            # ---------------- main loop over doc chunks ----------------
            pkq_tiles = []
            for h in range(DCHUNKS):
                dT = dpool.tile([128, 4 * 3 * 512], bf16, tag="demb")
                dT4 = dT[:].rearrange("p (b j n) -> p b j n", b=4, j=3)
                for beta in range(4):
                    di = dpool.tile(
                        [128, 512 // 16], i16, tag=f"didx{beta}", name=f"di{beta}"
                    )
                    nc.sync.dma_start(out=di[:], in_=d_idx[h, beta])
                    nc.gpsimd.dma_gather(
                        out_ap=dT4[:, beta],
                        in_ap=mtab[:],
                        idxs_ap=di[:],
                        num_idxs=512,
                        num_idxs_reg=512,
                        elem_size=MSLOT,
                        transpose=True,
                    )

                dtf = dpool.tile([4, 512], f32, tag="dtokf")
                nc.sync.dma_start(out=dtf[:], in_=d_tokf[h])

                cos = psum.tile([128, 512], f32, tag="cos")
                for beta in range(4):
                    b_glob = 4 * h + beta
                    g, qs = b_glob // 16, QPAD * (b_glob % 16)
                    nc.tensor.matmul(
                        out=cos[32 * beta : 32 * beta + 32, :],
                        lhsT=qT4[:, g, 0, qs : qs + QPAD],
                        rhs=dT4[:, beta, 0, :],
                        start=True,
                        stop=False,
                        tile_position=(0, 32 * beta),
                    )
                    nc.tensor.matmul(
                        out=cos[32 * beta : 32 * beta + 32, :],
                        lhsT=qT4[:, g, 1, qs : qs + QPAD],
                        rhs=dT4[:, beta, 1, :],
                        start=False,
                        stop=False,
                        tile_position=(0, 32 * beta),
                    )
                    nc.tensor.matmul(
                        out=cos[32 * beta : 32 * beta + 32, :],
                        lhsT=qT4[0:45, g, 2, qs : qs + QPAD],
                        rhs=dT4[0:45, beta, 2, :],
                        start=False,
                        stop=True,
                        tile_position=(0, 32 * beta),
                    )

                pkq = pkpool.tile([128, NK], f32, tag=f"pkq{h}")
                pkq_tiles.append(pkq)

                # k0: exact-token-match count (scaled by 2/sqrt(pi) so the
                # shared Ln(scale=sqrt(pi)/2) tail undoes it)
                ptb = psum.tile([128, 512], f32, tag="ptb")
                nc.tensor.matmul(
                    out=ptb[:],
                    lhsT=s_selT_t[:],
                    rhs=dtf[:],
                    start=True,
                    stop=True,
                )
                cmp = scr.tile([128, 512], f32, tag="cmp")
                nc.vector.tensor_scalar(
                    out=cmp[:],
                    in0=ptb[:],
                    scalar1=qtokf_t[:, h : h + 1],
                    scalar2=0.0,
                    op0=ALU.is_equal,
                    op1=ALU.add,
                    accum_out=pkq[:, 0:1],
                )

                if DEBUG:
                    cos_sb = scr.tile([128, 512], f32, tag="cossb", name="cos_sb")
                    nc.vector.tensor_copy(out=cos_sb[:], in_=cos[:])
                    nc.sync.dma_start(out=dbg_cos[h], in_=cos_sb[:])

                # Gaussian kernels 1..10: one derf pass each, accumulated
                # along the free (doc) dim straight out of PSUM
                sim = scr.tile([128, 512], f32, tag="sim")
                for k in range(1, NK):
                    nc.scalar.activation(
                        out=sim[:],
                        in_=cos[:],
                        func=AF.Derivative_Erf,
                        scale=SQRT50,
                        bias=derfb_t[:, k : k + 1],
                        accum_out=pkq[:, k : k + 1],
                    )

            if DEBUG:
                for h in range(DCHUNKS):
                    nc.sync.dma_start(out=dbg_pkq[h], in_=pkq_tiles[h][:])

            # ---------------- tail: clip, log, mask, reduce, dense ----------------
            out_acc = pkpool.tile([4, DCHUNKS], f32, tag="outacc")
            for h in range(DCHUNKS):
                pkq = pkq_tiles[h]
                # k0 is a raw count (no 2/sqrt(pi) factor): own clip + Ln scale
                nc.vector.tensor_scalar(
                    out=pkq[:, 0:1], in0=pkq[:, 0:1], scalar1=1e-10, scalar2=None,
                    op0=ALU.max,
                )
                nc.vector.tensor_scalar(
                    out=pkq[:, 1:NK], in0=pkq[:, 1:NK], scalar1=CLIP, scalar2=None,
                    op0=ALU.max,
                )
                lnp = pkpool.tile([128, NK], f32, tag=f"lnp{h}")
                nc.scalar.activation(
                    out=lnp[:, 0:1], in_=pkq[:, 0:1], func=AF.Ln, scale=1.0
                )
                nc.scalar.activation(
                    out=lnp[:, 1:NK], in_=pkq[:, 1:NK], func=AF.Ln, scale=LN_SCALE
                )
                nc.vector.tensor_scalar(
                    out=lnp[:],
                    in0=lnp[:],
                    scalar1=qm001_t[:, h : h + 1],
                    scalar2=None,
                    op0=ALU.mult,
                )
                pkp = psum.tile([4, NK], f32, tag="cos")
                nc.tensor.matmul(
                    out=pkp[:],
                    lhsT=s_sel_t[:],
                    rhs=lnp[:],
                    start=True,
                    stop=True,
                )
                pks = pkpool.tile([4, NK], f32, tag=f"pks{h}")
                nc.vector.tensor_tensor(
                    out=pks[:], in0=pkp[:], in1=w4_t[:], op=ALU.mult
                )
                nc.vector.reduce_sum(
                    out=out_acc[:, h : h + 1], in_=pks[:], axis=mybir.AxisListType.X
                )
            nc.scalar.activation(
                out=out_acc[:],
                in_=out_acc[:],
                func=AF.Identity,
                bias=b4_t[:, 0:1],
                scale=1.0,
            )
            nc.sync.dma_start(out=out[:], in_=out_acc[:])

    nc.compile()
    _prog_cache[key] = nc
    return nc


def _wrap16(idx, ncols):
    """[N] int16 -> [128, ncols] with idx i at [i%16, i//16], replicated
    across all eight 16-partition groups (Q7 channel copies)."""
    a = np.asarray(idx, dtype=np.int16).reshape(ncols, 16).T  # [16, ncols]
    return np.tile(a, (8, 1))


def _host_prep(query_tokens, doc_tokens, embed_table, dense_w, dense_b):
    import ml_dtypes

    emb = np.ascontiguousarray(embed_table, dtype=np.float32)
    norms = np.sqrt(np.sum(emb.astype(np.float64) ** 2, axis=1))
    n_emb = emb / np.maximum(norms, 1e-13).astype(np.float32)[:, None]

    qt = np.asarray(query_tokens).astype(np.int64)
    dt = np.asarray(doc_tokens).astype(np.int64)

    s_sel = np.zeros((128, 4), dtype=np.float32)
    for p in range(128):
        s_sel[p, p // 32] = 1.0

    derfb = np.tile(
        (-SQRT50 * np.asarray(MUS, dtype=np.float32)).reshape(1, NK), (128, 1)
    )

    in_maps = []
    for c in range(NCORES):
        qt_c = qt[c * BLOC : (c + 1) * BLOC]  # [32, 20]
        dt_c = dt[c * BLOC : (c + 1) * BLOC]  # [32, 512]
        q_pad = np.zeros((BLOC, QPAD), dtype=np.int64)
        q_pad[:, :Q] = qt_c
        qf = q_pad.reshape(-1)  # [1024] slot order 32b+i
        df = dt_c.reshape(-1)  # [16384]

        uniq = np.unique(np.concatenate([qf, df]))
        assert len(uniq) <= MROWS, len(uniq)
        mtab = np.zeros((MROWS, MSLOT), dtype=ml_dtypes.bfloat16)
        mtab[: len(uniq), :E] = n_emb[uniq].astype(ml_dtypes.bfloat16)
        z = np.searchsorted(uniq, 0)
        if z < len(uniq) and uniq[z] == 0:
            mtab[z, :E] = 0
            mtab[z, BIAS_COL] = MASK_BIAS

        q_i16 = np.searchsorted(uniq, qf).astype(np.int16)
        d_i16 = np.searchsorted(uniq, df).astype(np.int16)

        # one 512-idx gather per (chunk, batch): d_idx[h, beta] covers
        # batch 4h+beta's 512 doc tokens
        d_idx = np.stack(
            [
                np.stack(
                    [
                        _wrap16(
                            d_i16[(4 * h + beta) * 512 : (4 * h + beta + 1) * 512],
                            512 // 16,
                        )
                        for beta in range(4)
                    ]
                )
                for h in range(DCHUNKS)
            ]
        )
        q_idx = np.stack(
            [_wrap16(q_i16[g * 512 : (g + 1) * 512], 512 // 16) for g in range(2)]
        )

        qtokf = qf.reshape(DCHUNKS, 128).T.astype(np.float32)  # [128, 8]
        qm = (qf > 0).astype(np.float32) * 0.01
        qm001_a = qm.reshape(DCHUNKS, 128).T.astype(np.float32)
        d_tokf = dt_c.reshape(DCHUNKS, 4, 512).astype(np.float32)

        in_maps.append(
            {
                "mtab": mtab,
                "d_idx": d_idx,
                "q_idx": q_idx,
                "s_sel": s_sel,
                "s_selT": np.ascontiguousarray(s_sel.T),
                "d_tokf": d_tokf,
                "q_tokf": qtokf,
                "qm001": qm001_a,
                "w4": np.tile(
                    np.asarray(dense_w, dtype=np.float32).reshape(1, NK), (4, 1)
                ),
                "b4": np.full((4, 1), np.asarray(dense_b).reshape(-1)[0], np.float32),
                "derfb": derfb,
                "qones": np.ones((1, NQTOK), dtype=ml_dtypes.bfloat16),
            }
        )
    return in_maps


def _install_loud_hook():
    # surface exceptions raised inside the PJRT compile callback, which are
    # otherwise swallowed by the C++ layer
    import traceback
    from concourse import bass2jax

    if getattr(bass2jax, "_loud_hook_installed", False):
        return
    orig = bass2jax.neuronx_cc_hook

    def loud(*a, **k):
        try:
            return orig(*a, **k)
        except BaseException:
            traceback.print_exc()
            raise

    bass2jax.neuronx_cc_hook = loud
    bass2jax._loud_hook_installed = True


LAST_RESULTS = None


def kernel(query_tokens, doc_tokens, embed_table, dense_w, dense_b):
    global LAST_RESULTS
    _install_loud_hook()
    from concourse.bass_utils import run_bass_kernel_spmd

    nc = _build_program()
    in_maps = _host_prep(query_tokens, doc_tokens, embed_table, dense_w, dense_b)
    res = run_bass_kernel_spmd(nc, in_maps, list(range(NCORES)))
    LAST_RESULTS = res
    out = np.empty((B,), dtype=np.float32)
    for c in range(NCORES):
        arr = res.results[c]["out"]  # [4, 8]: batch 4h+beta at [beta, h]
        out[c * BLOC : (c + 1) * BLOC] = arr.T.reshape(BLOC)
    return out


# revision 31
# speedup vs baseline: 2.0626x; 1.3655x over previous
"""KNRM kernel for 8 Trainium2 NeuronCores (data-parallel over batch).

Per core (32 batches):
  - host: dedup this core's tokens (~16k unique < int16 max), build a
    pre-normalized bf16 mini-table [17472, 384] (300 emb dims + mask-bias
    column at 300: -1e6 for vocab id 0, else 0), remap token tensors to
    int16 mini-table indices replicated across 16-partition groups (the
    Q7 dma_gather ucode reads a copy per 16-partition channel group).
  - device: per 2048-token chunk, ONE dma_gather(transpose=True) delivers
    embeddings directly in [e, token] layout (partition p, free slot j
    holds element 128j+p), so the cosine matmuls need no PE transposes and
    no PSUM->SBUF copies. Masking is folded into the contraction via the
    bias column (query side forced to 1.0). Gaussian kernel pooling runs
    as ONE scalar-engine pass per kernel using Derivative_Erf
    (d/dx erf = 2/sqrt(pi) * exp(-x^2)) with free-dim accumulation,
    reading cos straight from PSUM; the 2/sqrt(pi) factor is undone by
    the Ln(scale=sqrt(pi)/2) in the tail. k0 (sigma=1e-4, exact token
    match) is a DVE token-equality count scaled by 2/sqrt(pi) to share
    the same tail.
"""

import sys

sys.path.insert(0, "/opt/trn_rl_repo")

import numpy as np

B, Q, D, V, E = 256, 20, 512, 100000, 300
NCORES = 8
BLOC = B // NCORES  # 32 batches per core
QPAD = 32  # query slots per batch (20 real + 12 pad)
NQTOK = BLOC * QPAD  # 1024 query gather slots per core
DCHUNKS = 8  # doc chunks per core
DCTOK = 2048  # doc tokens per chunk (= 4 batches)
NK = 11
MSLOT = 384  # mini-table row elems (bf16) -> 768B, 256B-multiple
MROWS = 17472  # >= max unique tokens per core (16384 doc + 1024 q)
BIAS_COL = 300
MASK_BIAS = -1.0e6

SQRT50 = float(np.sqrt(50.0))
LN_SCALE = float(np.sqrt(np.pi) / 2.0)  # undo derf's 2/sqrt(pi)
K0_SCALE = float(2.0 / np.sqrt(np.pi))
CLIP = 1e-10 / LN_SCALE


def _mus(n):
    l = [1.0]
    bs = 2.0 / (n - 1)
    l.append(1 - bs / 2)
    for i in range(1, n - 1):
        l.append(l[i] - bs)
    return l


MUS = _mus(NK)

_prog_cache = {}
DEBUG = False


def _build_program():
    key = ("nc", DEBUG)
    if key in _prog_cache:
        return _prog_cache[key]

    import concourse.bass as bass
    import concourse.bacc as bacc
    import concourse.mybir as mybir
    import concourse.tile as tile
    from concourse import library_config

    f32 = mybir.dt.float32
    bf16 = mybir.dt.bfloat16
    i16 = mybir.dt.int16
    AF = mybir.ActivationFunctionType
    ALU = mybir.AluOpType

    nc = bacc.Bacc(
        "TRN2",
        target_bir_lowering=False,
        debug=False,
        num_devices=NCORES,
        num_swdge_queues=4,
    )

    mtab = nc.dram_tensor("mtab", [MROWS, MSLOT], bf16, kind="ExternalInput").ap()
    d_idx = nc.dram_tensor(
        "d_idx", [DCHUNKS, 4, 128, 512 // 16], i16, kind="ExternalInput"
    ).ap()
    q_idx = nc.dram_tensor(
        "q_idx", [2, 128, 512 // 16], i16, kind="ExternalInput"
    ).ap()
    s_sel = nc.dram_tensor("s_sel", [128, 4], f32, kind="ExternalInput").ap()
    s_selT = nc.dram_tensor("s_selT", [4, 128], f32, kind="ExternalInput").ap()
    d_tokf = nc.dram_tensor(
        "d_tokf", [DCHUNKS, 4, 512], f32, kind="ExternalInput"
    ).ap()
    q_tokf = nc.dram_tensor("q_tokf", [128, DCHUNKS], f32, kind="ExternalInput").ap()
    qm88 = nc.dram_tensor(
        "qm88", [128, DCHUNKS * NK], f32, kind="ExternalInput"
    ).ap()
    w88 = nc.dram_tensor("w88", [4, DCHUNKS * NK], f32, kind="ExternalInput").ap()
    b4 = nc.dram_tensor("b4", [4, 1], f32, kind="ExternalInput").ap()
    derfb = nc.dram_tensor("derfb", [128, NK], f32, kind="ExternalInput").ap()
    qones = nc.dram_tensor("qones", [1, NQTOK], bf16, kind="ExternalInput").ap()
    out = nc.dram_tensor("out", [4, DCHUNKS], f32, kind="ExternalOutput").ap()
    dbg_pkq = (
        nc.dram_tensor("dbg_pkq", [DCHUNKS, 128, NK], f32, kind="ExternalOutput").ap()
        if DEBUG
        else None
    )
    dbg_cos = (
        nc.dram_tensor("dbg_cos", [DCHUNKS, 128, 512], f32, kind="ExternalOutput").ap()
        if DEBUG
        else None
    )

    with tile.TileContext(nc) as tc:
        import contextlib

        with contextlib.ExitStack() as ctx:
            const_pool = ctx.enter_context(tc.tile_pool(name="consts", bufs=1))
            qp = ctx.enter_context(tc.tile_pool(name="qprep", bufs=1))
            dpool = ctx.enter_context(tc.tile_pool(name="demb", bufs=2))
            pkpool = ctx.enter_context(tc.tile_pool(name="pk", bufs=1))
            scr = ctx.enter_context(tc.tile_pool(name="scr", bufs=2))
            psum = ctx.enter_context(
                tc.tile_pool(name="psum", bufs=2, space="PSUM")
            )

            nc.gpsimd.load_library(library_config.mlp)

            s_sel_t = const_pool.tile([128, 4], f32)
            nc.sync.dma_start(out=s_sel_t[:], in_=s_sel[:])
            s_selT_t = const_pool.tile([4, 128], f32)
            nc.sync.dma_start(out=s_selT_t[:], in_=s_selT[:])
            w88_t = const_pool.tile([4, DCHUNKS * NK], f32)
            nc.sync.dma_start(out=w88_t[:], in_=w88[:])
            b4_t = const_pool.tile([4, 1], f32)
            nc.sync.dma_start(out=b4_t[:], in_=b4[:])
            derfb_t = const_pool.tile([128, NK], f32)
            nc.sync.dma_start(out=derfb_t[:], in_=derfb[:])
            qtokf_t = const_pool.tile([128, DCHUNKS], f32)
            nc.sync.dma_start(out=qtokf_t[:], in_=q_tokf[:])
            qm88_t = const_pool.tile([128, DCHUNKS * NK], f32)
            nc.sync.dma_start(out=qm88_t[:], in_=qm88[:])

            # ---------------- Q gather (transposed), 2x512 idxs ----------------
            # queue_num must track the global SWDGE issue index mod 4 so that
            # Tile's round-robin DMA-sem lanes (mod 8) never mix queues: a
            # lane shared by two queues completes out of issue order and the
            # lane's wait threshold passes early (observed as stale dT reads)
            gather_i = 0
            qT = qp.tile([128, 2 * 3 * 512], bf16)
            qT4 = qT[:].rearrange("p (g j n) -> p g j n", g=2, j=3)
            for g in range(2):
                qi = qp.tile([128, 512 // 16], i16, tag=f"qi{g}", name=f"qi{g}")
                nc.sync.dma_start(out=qi[:], in_=q_idx[g])
                nc.gpsimd.dma_gather(
                    out_ap=qT4[:, g],
                    in_ap=mtab[:],
                    idxs_ap=qi[:],
                    num_idxs=512,
                    num_idxs_reg=512,
                    elem_size=MSLOT,
                    transpose=True,
                    queue_num=gather_i % 4,
                )
                gather_i += 1
                # query-side bias multiplier: force e-row 300 (tile 2, part 44)
                nc.sync.dma_start(
                    out=qT4[44:45, g : g + 1, 2, :], in_=qones[:, 512 * g : 512 * (g + 1)]
                )

            # ---------------- main loop over doc chunks ----------------
            # all chunks' pooled sums live in one [128, 8*11] tile: chunk h
            # owns columns 11h..11h+11 (k0 at 11h)
            pkq_all = pkpool.tile([128, DCHUNKS * NK], f32, tag="pkqall")
            for h in range(DCHUNKS):
                dT = dpool.tile([128, 4 * 3 * 512], bf16, tag="demb")
                dT4 = dT[:].rearrange("p (b j n) -> p b j n", b=4, j=3)
                for beta in range(4):
                    di = dpool.tile(
                        [128, 512 // 16], i16, tag=f"didx{beta}", name=f"di{beta}"
                    )
                    nc.sync.dma_start(out=di[:], in_=d_idx[h, beta])
                    nc.gpsimd.dma_gather(
                        out_ap=dT4[:, beta],
                        in_ap=mtab[:],
                        idxs_ap=di[:],
                        num_idxs=512,
                        num_idxs_reg=512,
                        elem_size=MSLOT,
                        transpose=True,
                        queue_num=gather_i % 4,
                    )
                    gather_i += 1

                dtf = dpool.tile([4, 512], f32, tag="dtokf")
                nc.sync.dma_start(out=dtf[:], in_=d_tokf[h])

                cos = psum.tile([128, 512], f32, tag="cos")
                for beta in range(4):
                    b_glob = 4 * h + beta
                    g, qs = b_glob // 16, QPAD * (b_glob % 16)
                    nc.tensor.matmul(
                        out=cos[32 * beta : 32 * beta + 32, :],
                        lhsT=qT4[:, g, 0, qs : qs + QPAD],
                        rhs=dT4[:, beta, 0, :],
                        start=True,
                        stop=False,
                        tile_position=(0, 32 * beta),
                    )
                    nc.tensor.matmul(
                        out=cos[32 * beta : 32 * beta + 32, :],
                        lhsT=qT4[:, g, 1, qs : qs + QPAD],
                        rhs=dT4[:, beta, 1, :],
                        start=False,
                        stop=False,
                        tile_position=(0, 32 * beta),
                    )
                    nc.tensor.matmul(
                        out=cos[32 * beta : 32 * beta + 32, :],
                        lhsT=qT4[0:45, g, 2, qs : qs + QPAD],
                        rhs=dT4[0:45, beta, 2, :],
                        start=False,
                        stop=True,
                        tile_position=(0, 32 * beta),
                    )

                pkq = pkq_all[:, NK * h : NK * (h + 1)]

                # k0: exact-token-match count (scaled by 2/sqrt(pi) so the
                # shared Ln(scale=sqrt(pi)/2) tail undoes it)
                ptb = psum.tile([128, 512], f32, tag="ptb")
                nc.tensor.matmul(
                    out=ptb[:],
                    lhsT=s_selT_t[:],
                    rhs=dtf[:],
                    start=True,
                    stop=True,
                )
                cmp = scr.tile([128, 512], f32, tag="cmp")
                nc.vector.tensor_scalar(
                    out=cmp[:],
                    in0=ptb[:],
                    scalar1=qtokf_t[:, h : h + 1],
                    scalar2=0.0,
                    op0=ALU.is_equal,
                    op1=ALU.add,
                    accum_out=pkq[:, 0:1],
                )

                if DEBUG:
                    cos_sb = scr.tile([128, 512], f32, tag="cossb", name="cos_sb")
                    nc.vector.tensor_copy(out=cos_sb[:], in_=cos[:])
                    nc.sync.dma_start(out=dbg_cos[h], in_=cos_sb[:])

                # Gaussian kernels 1..10: one derf pass each, accumulated
                # along the free (doc) dim straight out of PSUM
                sim = scr.tile([128, 512], f32, tag="sim")
                for k in range(1, NK):
                    nc.scalar.activation(
                        out=sim[:],
                        in_=cos[:],
                        func=AF.Derivative_Erf,
                        scale=SQRT50,
                        bias=derfb_t[:, k : k + 1],
                        accum_out=pkq[:, k : k + 1],
                    )

            if DEBUG:
                for h in range(DCHUNKS):
                    nc.sync.dma_start(
                        out=dbg_pkq[h], in_=pkq_all[:, NK * h : NK * (h + 1)]
                    )

            # ---------------- tail: clip, log, mask, reduce, dense ----------------
            # batched over all chunks: 2 clips, 2 Ln passes (k0 is a raw count,
            # no 2/sqrt(pi) factor -> own clip+scale), 1 mask-mult, 1 matmul
            pk3 = pkq_all[:].rearrange("p (h k) -> p h k", k=NK)
            nc.vector.tensor_scalar(
                out=pk3[:, :, 0:1], in0=pk3[:, :, 0:1], scalar1=1e-10, scalar2=None,
                op0=ALU.max,
            )
            nc.vector.tensor_scalar(
                out=pk3[:, :, 1:NK], in0=pk3[:, :, 1:NK], scalar1=CLIP, scalar2=None,
                op0=ALU.max,
            )
            lnp = pkpool.tile([128, DCHUNKS * NK], f32, tag="lnpall")
            ln3 = lnp[:].rearrange("p (h k) -> p h k", k=NK)
            nc.scalar.activation(
                out=ln3[:, :, 0:1], in_=pk3[:, :, 0:1], func=AF.Ln, scale=1.0
            )
            nc.scalar.activation(
                out=ln3[:, :, 1:NK], in_=pk3[:, :, 1:NK], func=AF.Ln, scale=LN_SCALE
            )
            nc.vector.tensor_tensor(
                out=lnp[:], in0=lnp[:], in1=qm88_t[:], op=ALU.mult
            )
            pkp = psum.tile([4, DCHUNKS * NK], f32, tag="pkp")
            nc.tensor.matmul(
                out=pkp[:],
                lhsT=s_sel_t[:],
                rhs=lnp[:],
                start=True,
                stop=True,
            )
            pks = pkpool.tile([4, DCHUNKS * NK], f32, tag="pks")
            nc.vector.tensor_tensor(
                out=pks[:], in0=pkp[:], in1=w88_t[:], op=ALU.mult
            )
            out_acc = pkpool.tile([4, DCHUNKS], f32, tag="outacc")
            pks3 = pks[:].rearrange("p (h k) -> p h k", k=NK)
            for h in range(DCHUNKS):
                nc.vector.reduce_sum(
                    out=out_acc[:, h : h + 1],
                    in_=pks3[:, h],
                    axis=mybir.AxisListType.X,
                )
            nc.scalar.activation(
                out=out_acc[:],
                in_=out_acc[:],
                func=AF.Identity,
                bias=b4_t[:, 0:1],
                scale=1.0,
            )
            nc.sync.dma_start(out=out[:], in_=out_acc[:])

    nc.compile()
    _prog_cache[key] = nc
    return nc


def _wrap16(idx, ncols):
    """[N] int16 -> [128, ncols] with idx i at [i%16, i//16], replicated
    across all eight 16-partition groups (Q7 channel copies)."""
    a = np.asarray(idx, dtype=np.int16).reshape(ncols, 16).T  # [16, ncols]
    return np.tile(a, (8, 1))


def _host_prep(query_tokens, doc_tokens, embed_table, dense_w, dense_b):
    import ml_dtypes

    emb = np.ascontiguousarray(embed_table, dtype=np.float32)
    norms = np.sqrt(np.sum(emb.astype(np.float64) ** 2, axis=1))
    n_emb = emb / np.maximum(norms, 1e-13).astype(np.float32)[:, None]

    qt = np.asarray(query_tokens).astype(np.int64)
    dt = np.asarray(doc_tokens).astype(np.int64)

    s_sel = np.zeros((128, 4), dtype=np.float32)
    for p in range(128):
        s_sel[p, p // 32] = 1.0

    derfb = np.tile(
        (-SQRT50 * np.asarray(MUS, dtype=np.float32)).reshape(1, NK), (128, 1)
    )

    in_maps = []
    for c in range(NCORES):
        qt_c = qt[c * BLOC : (c + 1) * BLOC]  # [32, 20]
        dt_c = dt[c * BLOC : (c + 1) * BLOC]  # [32, 512]
        q_pad = np.zeros((BLOC, QPAD), dtype=np.int64)
        q_pad[:, :Q] = qt_c
        qf = q_pad.reshape(-1)  # [1024] slot order 32b+i
        df = dt_c.reshape(-1)  # [16384]

        uniq = np.unique(np.concatenate([qf, df]))
        assert len(uniq) <= MROWS, len(uniq)
        mtab = np.zeros((MROWS, MSLOT), dtype=ml_dtypes.bfloat16)
        mtab[: len(uniq), :E] = n_emb[uniq].astype(ml_dtypes.bfloat16)
        z = np.searchsorted(uniq, 0)
        if z < len(uniq) and uniq[z] == 0:
            mtab[z, :E] = 0
            mtab[z, BIAS_COL] = MASK_BIAS

        q_i16 = np.searchsorted(uniq, qf).astype(np.int16)
        d_i16 = np.searchsorted(uniq, df).astype(np.int16)

        # one 512-idx gather per (chunk, batch): d_idx[h, beta] covers
        # batch 4h+beta's 512 doc tokens
        d_idx = np.stack(
            [
                np.stack(
                    [
                        _wrap16(
                            d_i16[(4 * h + beta) * 512 : (4 * h + beta + 1) * 512],
                            512 // 16,
                        )
                        for beta in range(4)
                    ]
                )
                for h in range(DCHUNKS)
            ]
        )
        q_idx = np.stack(
            [_wrap16(q_i16[g * 512 : (g + 1) * 512], 512 // 16) for g in range(2)]
        )

        qtokf = qf.reshape(DCHUNKS, 128).T.astype(np.float32)  # [128, 8]
        qm = (qf > 0).astype(np.float32) * 0.01
        qm001_a = qm.reshape(DCHUNKS, 128).T.astype(np.float32)
        qm88_a = np.repeat(qm001_a, NK, axis=1)  # [128, 88]
        d_tokf = dt_c.reshape(DCHUNKS, 4, 512).astype(np.float32)

        in_maps.append(
            {
                "mtab": mtab,
                "d_idx": d_idx,
                "q_idx": q_idx,
                "s_sel": s_sel,
                "s_selT": np.ascontiguousarray(s_sel.T),
                "d_tokf": d_tokf,
                "q_tokf": qtokf,
                "qm88": qm88_a,
                "w88": np.tile(
                    np.asarray(dense_w, dtype=np.float32).reshape(1, NK),
                    (4, DCHUNKS),
                ),
                "b4": np.full((4, 1), np.asarray(dense_b).reshape(-1)[0], np.float32),
                "derfb": derfb,
                "qones": np.ones((1, NQTOK), dtype=ml_dtypes.bfloat16),
            }
        )
    return in_maps


def _install_loud_hook():
    # surface exceptions raised inside the PJRT compile callback, which are
    # otherwise swallowed by the C++ layer
    import traceback
    from concourse import bass2jax

    if getattr(bass2jax, "_loud_hook_installed", False):
        return
    orig = bass2jax.neuronx_cc_hook

    def loud(*a, **k):
        try:
            return orig(*a, **k)
        except BaseException:
            traceback.print_exc()
            raise

    bass2jax.neuronx_cc_hook = loud
    bass2jax._loud_hook_installed = True


LAST_RESULTS = None


def kernel(query_tokens, doc_tokens, embed_table, dense_w, dense_b):
    global LAST_RESULTS
    _install_loud_hook()
    from concourse.bass_utils import run_bass_kernel_spmd

    nc = _build_program()
    in_maps = _host_prep(query_tokens, doc_tokens, embed_table, dense_w, dense_b)
    res = run_bass_kernel_spmd(nc, in_maps, list(range(NCORES)))
    LAST_RESULTS = res
    out = np.empty((B,), dtype=np.float32)
    for c in range(NCORES):
        arr = res.results[c]["out"]  # [4, 8]: batch 4h+beta at [beta, h]
        out[c * BLOC : (c + 1) * BLOC] = arr.T.reshape(BLOC)
    return out


# revision 40
# speedup vs baseline: 2.5695x; 1.2457x over previous
"""KNRM kernel for 8 Trainium2 NeuronCores (data-parallel over batch).

Per core (32 batches):
  - host: dedup this core's tokens (~16k unique < int16 max), build a
    pre-normalized bf16 mini-table [17472, 384] (300 emb dims + mask-bias
    column at 300: -1e6 for vocab id 0, else 0), remap token tensors to
    int16 mini-table indices replicated across 16-partition groups (the
    Q7 dma_gather ucode reads a copy per 16-partition channel group).
  - device: per 2048-token chunk, ONE dma_gather(transpose=True) delivers
    embeddings directly in [e, token] layout (partition p, free slot j
    holds element 128j+p), so the cosine matmuls need no PE transposes and
    no PSUM->SBUF copies. Masking is folded into the contraction via the
    bias column (query side forced to 1.0). Gaussian kernel pooling runs
    as ONE scalar-engine pass per kernel using Derivative_Erf
    (d/dx erf = 2/sqrt(pi) * exp(-x^2)) with free-dim accumulation,
    reading cos straight from PSUM; the 2/sqrt(pi) factor is undone by
    the Ln(scale=sqrt(pi)/2) in the tail. k0 (sigma=1e-4, exact token
    match) is a DVE token-equality count scaled by 2/sqrt(pi) to share
    the same tail.
"""

import sys

sys.path.insert(0, "/opt/trn_rl_repo")

import numpy as np

B, Q, D, V, E = 256, 20, 512, 100000, 300
NCORES = 8
BLOC = B // NCORES  # 32 batches per core
QPAD = 32  # query slots per batch (20 real + 12 pad)
NQTOK = BLOC * QPAD  # 1024 query gather slots per core
DCHUNKS = 8  # doc chunks per core
DCTOK = 2048  # doc tokens per chunk (= 4 batches)
NK = 11
MSLOT = 384  # mini-table row elems (bf16) -> 768B, 256B-multiple
MROWS = 17472  # >= max unique tokens per core (16384 doc + 1024 q)
BIAS_COL = 300
MASK_BIAS = -1.0e6

SQRT50 = float(np.sqrt(50.0))
LN_SCALE = float(np.sqrt(np.pi) / 2.0)  # undo derf's 2/sqrt(pi)
K0_SCALE = float(2.0 / np.sqrt(np.pi))
CLIP = 1e-10 / LN_SCALE
# Ln(LN_SCALE*x) applied to the raw k0 count leaves an extra ln(LN_SCALE);
# the correction (-ln(LN_SCALE) per valid q row) is folded into the dense
# bias on the host
LNC = float(-np.log(LN_SCALE))


def _mus(n):
    l = [1.0]
    bs = 2.0 / (n - 1)
    l.append(1 - bs / 2)
    for i in range(1, n - 1):
        l.append(l[i] - bs)
    return l


MUS = _mus(NK)

_prog_cache = {}
DEBUG = False


def _build_program():
    key = ("nc", DEBUG)
    if key in _prog_cache:
        return _prog_cache[key]

    import concourse.bass as bass
    import concourse.bacc as bacc
    import concourse.mybir as mybir
    import concourse.tile as tile
    from concourse import library_config

    f32 = mybir.dt.float32
    bf16 = mybir.dt.bfloat16
    i16 = mybir.dt.int16
    AF = mybir.ActivationFunctionType
    ALU = mybir.AluOpType

    nc = bacc.Bacc(
        "TRN2",
        target_bir_lowering=False,
        debug=False,
        num_devices=NCORES,
        num_swdge_queues=4,
    )

    mtab = nc.dram_tensor("mtab", [MROWS, MSLOT], bf16, kind="ExternalInput").ap()
    d_idx = nc.dram_tensor(
        "d_idx", [DCHUNKS, 4, 128, 512 // 16], i16, kind="ExternalInput"
    ).ap()
    q_idx = nc.dram_tensor(
        "q_idx", [2, 128, 512 // 16], i16, kind="ExternalInput"
    ).ap()
    s_sel = nc.dram_tensor("s_sel", [128, 4], f32, kind="ExternalInput").ap()
    s_selT = nc.dram_tensor("s_selT", [4, 128], f32, kind="ExternalInput").ap()
    d_tokf = nc.dram_tensor(
        "d_tokf", [DCHUNKS, 4, 512], f32, kind="ExternalInput"
    ).ap()
    q_tokf = nc.dram_tensor("q_tokf", [128, DCHUNKS], f32, kind="ExternalInput").ap()
    qm88 = nc.dram_tensor(
        "qm88", [128, DCHUNKS * NK], f32, kind="ExternalInput"
    ).ap()
    w88 = nc.dram_tensor("w88", [4, DCHUNKS * NK], f32, kind="ExternalInput").ap()
    beff = nc.dram_tensor("beff", [4, DCHUNKS], f32, kind="ExternalInput").ap()

    derfb = nc.dram_tensor("derfb", [128, NK], f32, kind="ExternalInput").ap()
    qones = nc.dram_tensor("qones", [1, NQTOK], bf16, kind="ExternalInput").ap()
    out = nc.dram_tensor("out", [4, DCHUNKS], f32, kind="ExternalOutput").ap()
    dbg_pkq = (
        nc.dram_tensor("dbg_pkq", [DCHUNKS, 128, NK], f32, kind="ExternalOutput").ap()
        if DEBUG
        else None
    )
    dbg_cos = (
        nc.dram_tensor("dbg_cos", [DCHUNKS, 128, 512], f32, kind="ExternalOutput").ap()
        if DEBUG
        else None
    )

    with tile.TileContext(nc) as tc:
        import contextlib

        with contextlib.ExitStack() as ctx:
            const_pool = ctx.enter_context(tc.tile_pool(name="consts", bufs=1))
            qp = ctx.enter_context(tc.tile_pool(name="qprep", bufs=1))
            dpool = ctx.enter_context(tc.tile_pool(name="demb", bufs=3))
            pkpool = ctx.enter_context(tc.tile_pool(name="pk", bufs=1))
            scr = ctx.enter_context(tc.tile_pool(name="scr", bufs=2))
            psum = ctx.enter_context(
                tc.tile_pool(name="psum", bufs=2, space="PSUM")
            )

            nc.gpsimd.load_library(library_config.mlp)

            s_sel_t = const_pool.tile([128, 4], f32)
            nc.sync.dma_start(out=s_sel_t[:], in_=s_sel[:])
            s_selT_t = const_pool.tile([4, 128], f32)
            nc.sync.dma_start(out=s_selT_t[:], in_=s_selT[:])
            w88_t = const_pool.tile([4, DCHUNKS * NK], f32)
            nc.sync.dma_start(out=w88_t[:], in_=w88[:])
            beff_t = const_pool.tile([4, DCHUNKS], f32)
            nc.sync.dma_start(out=beff_t[:], in_=beff[:])
            derfb_t = const_pool.tile([128, NK], f32)
            nc.sync.dma_start(out=derfb_t[:], in_=derfb[:])
            qtokf_t = const_pool.tile([128, DCHUNKS], f32)
            nc.sync.dma_start(out=qtokf_t[:], in_=q_tokf[:])
            qm88_t = const_pool.tile([128, DCHUNKS * NK], f32)
            nc.sync.dma_start(out=qm88_t[:], in_=qm88[:])

            # ---------------- Q gather (transposed), 2x512 idxs ----------------
            # queue_num must track the global SWDGE issue index mod 4 so that
            # Tile's round-robin DMA-sem lanes (mod 8) never mix queues: a
            # lane shared by two queues completes out of issue order and the
            # lane's wait threshold passes early (observed as stale dT reads)
            gather_i = 0
            qT = qp.tile([128, 2 * 3 * 512], bf16)
            qT4 = qT[:].rearrange("p (g j n) -> p g j n", g=2, j=3)
            for g in range(2):
                qi = qp.tile([128, 512 // 16], i16, tag=f"qi{g}", name=f"qi{g}")
                nc.sync.dma_start(out=qi[:], in_=q_idx[g])
                nc.gpsimd.dma_gather(
                    out_ap=qT4[:, g],
                    in_ap=mtab[:],
                    idxs_ap=qi[:],
                    num_idxs=512,
                    num_idxs_reg=512,
                    elem_size=MSLOT,
                    transpose=True,
                    queue_num=gather_i % 4,
                )
                gather_i += 1
                # query-side bias multiplier: force e-row 300 (tile 2, part 44)
                nc.sync.dma_start(
                    out=qT4[44:45, g : g + 1, 2, :], in_=qones[:, 512 * g : 512 * (g + 1)]
                )

            # ---------------- main loop over chunk pairs ----------------
            # all chunks' pooled sums live in one [128, 8*11] tile: chunk h
            # owns columns 11h..11h+11 (k0 at 11h)
            pkq_all = pkpool.tile([128, DCHUNKS * NK], f32, tag="pkqall")
            pk3 = pkq_all[:].rearrange("p (h k) -> p h k", k=NK)
            for grp in range(DCHUNKS // 2):
                # both chunks of the pair share one [128, 1024] PSUM cos tile
                # so each derf pass covers 1024 columns; the per-chunk sums
                # come from a segmented DVE reduce afterwards
                cos = psum.tile([128, 1024], f32, tag="cos")
                for j in range(2):
                    h = 2 * grp + j
                    dT = dpool.tile([128, 4 * 3 * 512], bf16, tag="demb")
                    dT4 = dT[:].rearrange("p (b j n) -> p b j n", b=4, j=3)
                    for beta in range(4):
                        di = dpool.tile(
                            [128, 512 // 16], i16, tag=f"didx{beta}", name=f"di{beta}"
                        )
                        nc.sync.dma_start(out=di[:], in_=d_idx[h, beta])
                        nc.gpsimd.dma_gather(
                            out_ap=dT4[:, beta],
                            in_ap=mtab[:],
                            idxs_ap=di[:],
                            num_idxs=512,
                            num_idxs_reg=512,
                            elem_size=MSLOT,
                            transpose=True,
                            queue_num=gather_i % 4,
                        )
                        gather_i += 1

                    dtf = dpool.tile([4, 512], f32, tag="dtokf")
                    nc.sync.dma_start(out=dtf[:], in_=d_tokf[h])

                    for beta in range(4):
                        b_glob = 4 * h + beta
                        g, qs = b_glob // 16, QPAD * (b_glob % 16)
                        cob = cos[32 * beta : 32 * beta + 32, 512 * j : 512 * (j + 1)]
                        nc.tensor.matmul(
                            out=cob,
                            lhsT=qT4[:, g, 0, qs : qs + QPAD],
                            rhs=dT4[:, beta, 0, :],
                            start=True,
                            stop=False,
                            tile_position=(0, 32 * beta),
                        )
                        nc.tensor.matmul(
                            out=cob,
                            lhsT=qT4[:, g, 1, qs : qs + QPAD],
                            rhs=dT4[:, beta, 1, :],
                            start=False,
                            stop=False,
                            tile_position=(0, 32 * beta),
                        )
                        nc.tensor.matmul(
                            out=cob,
                            lhsT=qT4[0:45, g, 2, qs : qs + QPAD],
                            rhs=dT4[0:45, beta, 2, :],
                            start=False,
                            stop=True,
                            tile_position=(0, 32 * beta),
                        )

                    # k0: exact-token-match count
                    ptb = psum.tile([128, 512], f32, tag="ptb")
                    nc.tensor.matmul(
                        out=ptb[:],
                        lhsT=s_selT_t[:],
                        rhs=dtf[:],
                        start=True,
                        stop=True,
                    )
                    cmp = scr.tile([128, 512], f32, tag="cmp")
                    nc.vector.tensor_scalar(
                        out=cmp[:],
                        in0=ptb[:],
                        scalar1=qtokf_t[:, h : h + 1],
                        scalar2=0.0,
                        op0=ALU.is_equal,
                        op1=ALU.add,
                        accum_out=pk3[:, h, 0:1],
                    )

                    if DEBUG:
                        cos_sb = scr.tile([128, 512], f32, tag="cossb", name="cos_sb")
                        nc.vector.tensor_copy(
                            out=cos_sb[:], in_=cos[:, 512 * j : 512 * (j + 1)]
                        )
                        nc.sync.dma_start(out=dbg_cos[h], in_=cos_sb[:])

                # Gaussian kernels 1..10: one derf pass over both chunks, then
                # a segmented reduce for the two per-chunk sums
                for k in range(1, NK):
                    sim = scr.tile([128, 1024], f32, tag="sim")
                    nc.scalar.activation(
                        out=sim[:],
                        in_=cos[:],
                        func=AF.Derivative_Erf,
                        scale=SQRT50,
                        bias=derfb_t[:, k : k + 1],
                    )
                    nc.vector.tensor_reduce(
                        out=pk3[:, 2 * grp : 2 * grp + 2, k : k + 1],
                        in_=sim[:].rearrange("p (c n) -> p c n", c=2),
                        axis=mybir.AxisListType.X,
                        op=ALU.add,
                    )

            if DEBUG:
                for h in range(DCHUNKS):
                    nc.sync.dma_start(
                        out=dbg_pkq[h], in_=pkq_all[:, NK * h : NK * (h + 1)]
                    )

            # ---------------- tail: clip, log, mask, reduce, dense ----------------
            # batched over all chunks: 2 clips, 2 Ln passes (k0 is a raw count,
            # no 2/sqrt(pi) factor -> own clip+scale), 1 mask-mult, 1 matmul
            pk3 = pkq_all[:].rearrange("p (h k) -> p h k", k=NK)
            nc.vector.tensor_scalar(
                out=pk3[:, :, 0:1], in0=pk3[:, :, 0:1], scalar1=1e-10, scalar2=None,
                op0=ALU.max,
            )
            nc.vector.tensor_scalar(
                out=pk3[:, :, 1:NK], in0=pk3[:, :, 1:NK], scalar1=CLIP, scalar2=None,
                op0=ALU.max,
            )
            # one Ln for all 88 columns (forced after the last derf since it
            # reads the whole pkq tile -> exactly 2 ACT table loads per run);
            # the spurious ln(LN_SCALE) this adds to the k0 columns is undone
            # via the host-folded dense bias (beff)
            lnp = pkpool.tile([128, DCHUNKS * NK], f32, tag="lnpall")
            nc.scalar.activation(
                out=lnp[:], in_=pkq_all[:], func=AF.Ln, scale=LN_SCALE
            )
            nc.vector.tensor_tensor(
                out=lnp[:], in0=lnp[:], in1=qm88_t[:], op=ALU.mult
            )
            pkp = psum.tile([4, DCHUNKS * NK], f32, tag="pkp")
            nc.tensor.matmul(
                out=pkp[:],
                lhsT=s_sel_t[:],
                rhs=lnp[:],
                start=True,
                stop=True,
            )
            pks = pkpool.tile([4, DCHUNKS * NK], f32, tag="pks")
            nc.vector.tensor_tensor(
                out=pks[:], in0=pkp[:], in1=w88_t[:], op=ALU.mult
            )
            out_acc = pkpool.tile([4, DCHUNKS], f32, tag="outacc")
            pks3 = pks[:].rearrange("p (h k) -> p h k", k=NK)
            for h in range(DCHUNKS):
                nc.vector.reduce_sum(
                    out=out_acc[:, h : h + 1],
                    in_=pks3[:, h],
                    axis=mybir.AxisListType.X,
                )
            nc.vector.tensor_tensor(
                out=out_acc[:], in0=out_acc[:], in1=beff_t[:], op=ALU.add
            )
            nc.sync.dma_start(out=out[:], in_=out_acc[:])

    nc.compile()
    _prog_cache[key] = nc
    return nc


def _wrap16(idx, ncols):
    """[N] int16 -> [128, ncols] with idx i at [i%16, i//16], replicated
    across all eight 16-partition groups (Q7 channel copies)."""
    a = np.asarray(idx, dtype=np.int16).reshape(ncols, 16).T  # [16, ncols]
    return np.tile(a, (8, 1))


def _host_prep(query_tokens, doc_tokens, embed_table, dense_w, dense_b):
    import ml_dtypes

    emb = np.ascontiguousarray(embed_table, dtype=np.float32)
    norms = np.sqrt(np.sum(emb.astype(np.float64) ** 2, axis=1))
    n_emb = emb / np.maximum(norms, 1e-13).astype(np.float32)[:, None]

    qt = np.asarray(query_tokens).astype(np.int64)
    dt = np.asarray(doc_tokens).astype(np.int64)

    s_sel = np.zeros((128, 4), dtype=np.float32)
    for p in range(128):
        s_sel[p, p // 32] = 1.0

    derfb = np.tile(
        (-SQRT50 * np.asarray(MUS, dtype=np.float32)).reshape(1, NK), (128, 1)
    )

    in_maps = []
    for c in range(NCORES):
        qt_c = qt[c * BLOC : (c + 1) * BLOC]  # [32, 20]
        dt_c = dt[c * BLOC : (c + 1) * BLOC]  # [32, 512]
        q_pad = np.zeros((BLOC, QPAD), dtype=np.int64)
        q_pad[:, :Q] = qt_c
        qf = q_pad.reshape(-1)  # [1024] slot order 32b+i
        df = dt_c.reshape(-1)  # [16384]

        uniq = np.unique(np.concatenate([qf, df]))
        assert len(uniq) <= MROWS, len(uniq)
        mtab = np.zeros((MROWS, MSLOT), dtype=ml_dtypes.bfloat16)
        mtab[: len(uniq), :E] = n_emb[uniq].astype(ml_dtypes.bfloat16)
        z = np.searchsorted(uniq, 0)
        if z < len(uniq) and uniq[z] == 0:
            mtab[z, :E] = 0
            mtab[z, BIAS_COL] = MASK_BIAS

        q_i16 = np.searchsorted(uniq, qf).astype(np.int16)
        d_i16 = np.searchsorted(uniq, df).astype(np.int16)

        # one 512-idx gather per (chunk, batch): d_idx[h, beta] covers
        # batch 4h+beta's 512 doc tokens
        d_idx = np.stack(
            [
                np.stack(
                    [
                        _wrap16(
                            d_i16[(4 * h + beta) * 512 : (4 * h + beta + 1) * 512],
                            512 // 16,
                        )
                        for beta in range(4)
                    ]
                )
                for h in range(DCHUNKS)
            ]
        )
        q_idx = np.stack(
            [_wrap16(q_i16[g * 512 : (g + 1) * 512], 512 // 16) for g in range(2)]
        )

        qtokf = qf.reshape(DCHUNKS, 128).T.astype(np.float32)  # [128, 8]
        qm = (qf > 0).astype(np.float32) * 0.01
        qm001_a = qm.reshape(DCHUNKS, 128).T.astype(np.float32)
        qm88_a = np.repeat(qm001_a, NK, axis=1)  # [128, 88]
        d_tokf = dt_c.reshape(DCHUNKS, 4, 512).astype(np.float32)

        in_maps.append(
            {
                "mtab": mtab,
                "d_idx": d_idx,
                "q_idx": q_idx,
                "s_sel": s_sel,
                "s_selT": np.ascontiguousarray(s_sel.T),
                "d_tokf": d_tokf,
                "q_tokf": qtokf,
                "qm88": qm88_a,
                "w88": np.tile(
                    np.asarray(dense_w, dtype=np.float32).reshape(1, NK),
                    (4, DCHUNKS),
                ),
                "beff": (
                    np.asarray(dense_b, np.float32).reshape(-1)[0]
                    + np.asarray(dense_w, np.float32).reshape(-1)[0]
                    * 0.01
                    * LNC
                    * (qt_c > 0).sum(axis=1).reshape(DCHUNKS, 4).T
                ).astype(np.float32),
                "derfb": derfb,
                "qones": np.ones((1, NQTOK), dtype=ml_dtypes.bfloat16),
            }
        )
    return in_maps


def _install_loud_hook():
    # surface exceptions raised inside the PJRT compile callback, which are
    # otherwise swallowed by the C++ layer
    import traceback
    from concourse import bass2jax

    if getattr(bass2jax, "_loud_hook_installed", False):
        return
    orig = bass2jax.neuronx_cc_hook

    def loud(*a, **k):
        try:
            return orig(*a, **k)
        except BaseException:
            traceback.print_exc()
            raise

    bass2jax.neuronx_cc_hook = loud
    bass2jax._loud_hook_installed = True


LAST_RESULTS = None


def kernel(query_tokens, doc_tokens, embed_table, dense_w, dense_b):
    global LAST_RESULTS
    _install_loud_hook()
    from concourse.bass_utils import run_bass_kernel_spmd

    nc = _build_program()
    in_maps = _host_prep(query_tokens, doc_tokens, embed_table, dense_w, dense_b)
    res = run_bass_kernel_spmd(nc, in_maps, list(range(NCORES)))
    LAST_RESULTS = res
    out = np.empty((B,), dtype=np.float32)
    for c in range(NCORES):
        arr = res.results[c]["out"]  # [4, 8]: batch 4h+beta at [beta, h]
        out[c * BLOC : (c + 1) * BLOC] = arr.T.reshape(BLOC)
    return out


# revision 49
# speedup vs baseline: 2.5790x; 1.0037x over previous
"""KNRM kernel for 8 Trainium2 NeuronCores (data-parallel over batch).

Per core (32 batches):
  - host: dedup this core's tokens (~16k unique < int16 max), build a
    pre-normalized bf16 mini-table [17472, 384] (300 emb dims + mask-bias
    column at 300: -1e6 for vocab id 0, else 0), remap token tensors to
    int16 mini-table indices replicated across 16-partition groups (the
    Q7 dma_gather ucode reads a copy per 16-partition channel group).
  - device: per 2048-token chunk, ONE dma_gather(transpose=True) delivers
    embeddings directly in [e, token] layout (partition p, free slot j
    holds element 128j+p), so the cosine matmuls need no PE transposes and
    no PSUM->SBUF copies. Masking is folded into the contraction via the
    bias column (query side forced to 1.0). Gaussian kernel pooling runs
    as ONE scalar-engine pass per kernel using Derivative_Erf
    (d/dx erf = 2/sqrt(pi) * exp(-x^2)) with free-dim accumulation,
    reading cos straight from PSUM; the 2/sqrt(pi) factor is undone by
    the Ln(scale=sqrt(pi)/2) in the tail. k0 (sigma=1e-4, exact token
    match) is a DVE token-equality count scaled by 2/sqrt(pi) to share
    the same tail.
"""

import sys

sys.path.insert(0, "/opt/trn_rl_repo")

import numpy as np

B, Q, D, V, E = 256, 20, 512, 100000, 300
NCORES = 8
BLOC = B // NCORES  # 32 batches per core
QPAD = 32  # query slots per batch (20 real + 12 pad)
NQTOK = BLOC * QPAD  # 1024 query gather slots per core
DCHUNKS = 8  # doc chunks per core
DCTOK = 2048  # doc tokens per chunk (= 4 batches)
NK = 11
MSLOT = 384  # mini-table row elems (bf16) -> 768B, 256B-multiple
MROWS = 17472  # >= max unique tokens per core (16384 doc + 1024 q)
BIAS_COL = 300
MASK_BIAS = -1.0e6

SQRT50 = float(np.sqrt(50.0))
LN_SCALE = float(np.sqrt(np.pi) / 2.0)  # undo derf's 2/sqrt(pi)
K0_SCALE = float(2.0 / np.sqrt(np.pi))
CLIP = 1e-10 / LN_SCALE
# Ln(LN_SCALE*x) applied to the raw k0 count leaves an extra ln(LN_SCALE);
# the correction (-ln(LN_SCALE) per valid q row) is folded into the dense
# bias on the host
LNC = float(-np.log(LN_SCALE))


def _mus(n):
    l = [1.0]
    bs = 2.0 / (n - 1)
    l.append(1 - bs / 2)
    for i in range(1, n - 1):
        l.append(l[i] - bs)
    return l


MUS = _mus(NK)

_prog_cache = {}
DEBUG = False


def _build_program():
    key = ("nc", DEBUG)
    if key in _prog_cache:
        return _prog_cache[key]

    import concourse.bass as bass
    import concourse.bacc as bacc
    import concourse.mybir as mybir
    import concourse.tile as tile
    from concourse import library_config

    f32 = mybir.dt.float32
    bf16 = mybir.dt.bfloat16
    i16 = mybir.dt.int16
    AF = mybir.ActivationFunctionType
    ALU = mybir.AluOpType

    nc = bacc.Bacc(
        "TRN2",
        target_bir_lowering=False,
        debug=False,
        num_devices=NCORES,
        num_swdge_queues=4,
    )

    mtab = nc.dram_tensor("mtab", [MROWS, MSLOT], bf16, kind="ExternalInput").ap()
    d_idx = nc.dram_tensor(
        "d_idx", [128, DCHUNKS * 4 * (512 // 16)], i16, kind="ExternalInput"
    ).ap()
    q_idx = nc.dram_tensor(
        "q_idx", [2, 128, 512 // 16], i16, kind="ExternalInput"
    ).ap()
    s_sel = nc.dram_tensor("s_sel", [128, 4], f32, kind="ExternalInput").ap()
    s_selT = nc.dram_tensor("s_selT", [4, 128], f32, kind="ExternalInput").ap()
    d_tokf = nc.dram_tensor(
        "d_tokf", [4, DCHUNKS * 512], f32, kind="ExternalInput"
    ).ap()
    q_tokf = nc.dram_tensor("q_tokf", [128, DCHUNKS], f32, kind="ExternalInput").ap()
    qm88 = nc.dram_tensor(
        "qm88", [128, DCHUNKS * NK], f32, kind="ExternalInput"
    ).ap()
    w88 = nc.dram_tensor("w88", [4, DCHUNKS * NK], f32, kind="ExternalInput").ap()
    beff = nc.dram_tensor("beff", [4, DCHUNKS], f32, kind="ExternalInput").ap()

    derfb = nc.dram_tensor("derfb", [128, NK], f32, kind="ExternalInput").ap()
    qones = nc.dram_tensor("qones", [1, NQTOK], bf16, kind="ExternalInput").ap()
    out = nc.dram_tensor("out", [4, DCHUNKS], f32, kind="ExternalOutput").ap()
    dbg_pkq = (
        nc.dram_tensor("dbg_pkq", [DCHUNKS, 128, NK], f32, kind="ExternalOutput").ap()
        if DEBUG
        else None
    )
    dbg_cos = (
        nc.dram_tensor("dbg_cos", [DCHUNKS, 128, 512], f32, kind="ExternalOutput").ap()
        if DEBUG
        else None
    )

    with tile.TileContext(nc) as tc:
        import contextlib

        with contextlib.ExitStack() as ctx:
            const_pool = ctx.enter_context(tc.tile_pool(name="consts", bufs=1))
            qp = ctx.enter_context(tc.tile_pool(name="qprep", bufs=1))
            dpool = ctx.enter_context(tc.tile_pool(name="demb", bufs=3))
            pkpool = ctx.enter_context(tc.tile_pool(name="pk", bufs=1))
            scr = ctx.enter_context(tc.tile_pool(name="scr", bufs=2))
            psum = ctx.enter_context(
                tc.tile_pool(name="psum", bufs=2, space="PSUM")
            )

            nc.gpsimd.load_library(library_config.mlp)

            s_sel_t = const_pool.tile([128, 4], f32)
            nc.sync.dma_start(out=s_sel_t[:], in_=s_sel[:])
            s_selT_t = const_pool.tile([4, 128], f32)
            nc.sync.dma_start(out=s_selT_t[:], in_=s_selT[:])
            w88_t = const_pool.tile([4, DCHUNKS * NK], f32)
            nc.sync.dma_start(out=w88_t[:], in_=w88[:])
            beff_t = const_pool.tile([4, DCHUNKS], f32)
            nc.sync.dma_start(out=beff_t[:], in_=beff[:])
            derfb_t = const_pool.tile([128, NK], f32)
            nc.sync.dma_start(out=derfb_t[:], in_=derfb[:])
            qtokf_t = const_pool.tile([128, DCHUNKS], f32)
            nc.sync.dma_start(out=qtokf_t[:], in_=q_tokf[:])
            qm88_t = const_pool.tile([128, DCHUNKS * NK], f32)
            nc.sync.dma_start(out=qm88_t[:], in_=qm88[:])

            # ---------------- Q gather (transposed), 2x512 idxs ----------------
            # queue_num must track the global SWDGE issue index mod 4 so that
            # Tile's round-robin DMA-sem lanes (mod 8) never mix queues: a
            # lane shared by two queues completes out of issue order and the
            # lane's wait threshold passes early (observed as stale dT reads)
            gather_i = 0
            qT = qp.tile([128, 2 * 3 * 512], bf16)
            qT4 = qT[:].rearrange("p (g j n) -> p g j n", g=2, j=3)
            for g in range(2):
                qi = qp.tile([128, 512 // 16], i16, tag=f"qi{g}", name=f"qi{g}")
                nc.sync.dma_start(out=qi[:], in_=q_idx[g])
                nc.gpsimd.dma_gather(
                    out_ap=qT4[:, g],
                    in_ap=mtab[:],
                    idxs_ap=qi[:],
                    num_idxs=512,
                    num_idxs_reg=512,
                    elem_size=MSLOT,
                    transpose=True,
                    queue_num=gather_i % 4,
                )
                gather_i += 1
                # query-side bias multiplier: force e-row 300 (tile 2, part 44)
                nc.sync.dma_start(
                    out=qT4[44:45, g : g + 1, 2, :], in_=qones[:, 512 * g : 512 * (g + 1)]
                )

            # all doc-gather indices and token floats land in two upfront DMAs
            # (host pre-arranged to these layouts)
            di_all = qp.tile([128, DCHUNKS * 4 * (512 // 16)], i16)
            di4 = di_all[:].rearrange("p (h b n) -> p h b n", h=DCHUNKS, b=4)
            nc.sync.dma_start(out=di_all[:], in_=d_idx[:])
            dtf_all = qp.tile([4, DCHUNKS * 512], f32)
            dtf3 = dtf_all[:].rearrange("p (h n) -> p h n", h=DCHUNKS)
            nc.sync.dma_start(out=dtf_all[:], in_=d_tokf[:])

            # ---------------- main loop over chunk pairs ----------------
            # all chunks' pooled sums live in one [128, 8*11] tile: chunk h
            # owns columns 11h..11h+11 (k0 at 11h)
            pkq_all = pkpool.tile([128, DCHUNKS * NK], f32, tag="pkqall")
            pk3 = pkq_all[:].rearrange("p (h k) -> p h k", k=NK)
            for grp in range(DCHUNKS // 2):
                # both chunks of the pair share one [128, 1024] PSUM cos tile
                # so each derf pass covers 1024 columns; the per-chunk sums
                # come from a segmented DVE reduce afterwards
                cos = psum.tile([128, 1024], f32, tag="cos")
                for j in range(2):
                    h = 2 * grp + j
                    dT = dpool.tile([128, 4 * 3 * 512], bf16, tag="demb")
                    dT4 = dT[:].rearrange("p (b j n) -> p b j n", b=4, j=3)
                    for beta in range(4):
                        nc.gpsimd.dma_gather(
                            out_ap=dT4[:, beta],
                            in_ap=mtab[:],
                            idxs_ap=di4[:, h, beta],
                            num_idxs=512,
                            num_idxs_reg=512,
                            elem_size=MSLOT,
                            transpose=True,
                            queue_num=gather_i % 4,
                        )
                        gather_i += 1

                    dtf = dtf3[:, h]

                    for beta in range(4):
                        b_glob = 4 * h + beta
                        g, qs = b_glob // 16, QPAD * (b_glob % 16)
                        cob = cos[32 * beta : 32 * beta + 32, 512 * j : 512 * (j + 1)]
                        nc.tensor.matmul(
                            out=cob,
                            lhsT=qT4[:, g, 0, qs : qs + QPAD],
                            rhs=dT4[:, beta, 0, :],
                            start=True,
                            stop=False,
                            tile_position=(0, 32 * beta),
                        )
                        nc.tensor.matmul(
                            out=cob,
                            lhsT=qT4[:, g, 1, qs : qs + QPAD],
                            rhs=dT4[:, beta, 1, :],
                            start=False,
                            stop=False,
                            tile_position=(0, 32 * beta),
                        )
                        nc.tensor.matmul(
                            out=cob,
                            lhsT=qT4[0:45, g, 2, qs : qs + QPAD],
                            rhs=dT4[0:45, beta, 2, :],
                            start=False,
                            stop=True,
                            tile_position=(0, 32 * beta),
                        )

                    # k0: exact-token-match count
                    ptb = psum.tile([128, 512], f32, tag="ptb")
                    nc.tensor.matmul(
                        out=ptb[:],
                        lhsT=s_selT_t[:],
                        rhs=dtf,
                        start=True,
                        stop=True,
                    )
                    cmp = scr.tile([128, 512], f32, tag="cmp")
                    nc.vector.tensor_scalar(
                        out=cmp[:],
                        in0=ptb[:],
                        scalar1=qtokf_t[:, h : h + 1],
                        scalar2=0.0,
                        op0=ALU.is_equal,
                        op1=ALU.add,
                        accum_out=pk3[:, h, 0:1],
                    )

                    if DEBUG:
                        cos_sb = scr.tile([128, 512], f32, tag="cossb", name="cos_sb")
                        nc.vector.tensor_copy(
                            out=cos_sb[:], in_=cos[:, 512 * j : 512 * (j + 1)]
                        )
                        nc.sync.dma_start(out=dbg_cos[h], in_=cos_sb[:])

                # Gaussian kernels 1..10: one derf pass over both chunks, then
                # a segmented reduce for the two per-chunk sums
                for k in range(1, NK):
                    sim = scr.tile([128, 1024], bf16, tag="sim")
                    nc.scalar.activation(
                        out=sim[:],
                        in_=cos[:],
                        func=AF.Derivative_Erf,
                        scale=SQRT50,
                        bias=derfb_t[:, k : k + 1],
                    )
                    nc.vector.tensor_reduce(
                        out=pk3[:, 2 * grp : 2 * grp + 2, k : k + 1],
                        in_=sim[:].rearrange("p (c n) -> p c n", c=2),
                        axis=mybir.AxisListType.X,
                        op=ALU.add,
                    )

            if DEBUG:
                for h in range(DCHUNKS):
                    nc.sync.dma_start(
                        out=dbg_pkq[h], in_=pkq_all[:, NK * h : NK * (h + 1)]
                    )

            # ---------------- tail: clip, log, mask, reduce, dense ----------------
            # batched over all chunks: 2 clips, 2 Ln passes (k0 is a raw count,
            # no 2/sqrt(pi) factor -> own clip+scale), 1 mask-mult, 1 matmul
            pk3 = pkq_all[:].rearrange("p (h k) -> p h k", k=NK)
            nc.vector.tensor_scalar(
                out=pk3[:, :, 0:1], in0=pk3[:, :, 0:1], scalar1=1e-10, scalar2=None,
                op0=ALU.max,
            )
            nc.vector.tensor_scalar(
                out=pk3[:, :, 1:NK], in0=pk3[:, :, 1:NK], scalar1=CLIP, scalar2=None,
                op0=ALU.max,
            )
            # one Ln for all 88 columns (forced after the last derf since it
            # reads the whole pkq tile -> exactly 2 ACT table loads per run);
            # the spurious ln(LN_SCALE) this adds to the k0 columns is undone
            # via the host-folded dense bias (beff)
            lnp = pkpool.tile([128, DCHUNKS * NK], f32, tag="lnpall")
            nc.scalar.activation(
                out=lnp[:], in_=pkq_all[:], func=AF.Ln, scale=LN_SCALE
            )
            nc.vector.tensor_tensor(
                out=lnp[:], in0=lnp[:], in1=qm88_t[:], op=ALU.mult
            )
            pkp = psum.tile([4, DCHUNKS * NK], f32, tag="pkp")
            nc.tensor.matmul(
                out=pkp[:],
                lhsT=s_sel_t[:],
                rhs=lnp[:],
                start=True,
                stop=True,
            )
            pks = pkpool.tile([4, DCHUNKS * NK], f32, tag="pks")
            nc.vector.tensor_tensor(
                out=pks[:], in0=pkp[:], in1=w88_t[:], op=ALU.mult
            )
            out_acc = pkpool.tile([4, DCHUNKS], f32, tag="outacc")
            pks3 = pks[:].rearrange("p (h k) -> p h k", k=NK)
            for h in range(DCHUNKS):
                nc.vector.reduce_sum(
                    out=out_acc[:, h : h + 1],
                    in_=pks3[:, h],
                    axis=mybir.AxisListType.X,
                )
            nc.vector.tensor_tensor(
                out=out_acc[:], in0=out_acc[:], in1=beff_t[:], op=ALU.add
            )
            nc.sync.dma_start(out=out[:], in_=out_acc[:])

    nc.compile()
    _prog_cache[key] = nc
    return nc


def _wrap16(idx, ncols):
    """[N] int16 -> [128, ncols] with idx i at [i%16, i//16], replicated
    across all eight 16-partition groups (Q7 channel copies)."""
    a = np.asarray(idx, dtype=np.int16).reshape(ncols, 16).T  # [16, ncols]
    return np.tile(a, (8, 1))


def _host_prep(query_tokens, doc_tokens, embed_table, dense_w, dense_b):
    import ml_dtypes

    emb = np.ascontiguousarray(embed_table, dtype=np.float32)
    norms = np.sqrt(np.sum(emb.astype(np.float64) ** 2, axis=1))
    n_emb = emb / np.maximum(norms, 1e-13).astype(np.float32)[:, None]

    qt = np.asarray(query_tokens).astype(np.int64)
    dt = np.asarray(doc_tokens).astype(np.int64)

    s_sel = np.zeros((128, 4), dtype=np.float32)
    for p in range(128):
        s_sel[p, p // 32] = 1.0

    derfb = np.tile(
        (-SQRT50 * np.asarray(MUS, dtype=np.float32)).reshape(1, NK), (128, 1)
    )

    in_maps = []
    for c in range(NCORES):
        qt_c = qt[c * BLOC : (c + 1) * BLOC]  # [32, 20]
        dt_c = dt[c * BLOC : (c + 1) * BLOC]  # [32, 512]
        q_pad = np.zeros((BLOC, QPAD), dtype=np.int64)
        q_pad[:, :Q] = qt_c
        qf = q_pad.reshape(-1)  # [1024] slot order 32b+i
        df = dt_c.reshape(-1)  # [16384]

        uniq = np.unique(np.concatenate([qf, df]))
        assert len(uniq) <= MROWS, len(uniq)
        mtab = np.zeros((MROWS, MSLOT), dtype=ml_dtypes.bfloat16)
        mtab[: len(uniq), :E] = n_emb[uniq].astype(ml_dtypes.bfloat16)
        z = np.searchsorted(uniq, 0)
        if z < len(uniq) and uniq[z] == 0:
            mtab[z, :E] = 0
            mtab[z, BIAS_COL] = MASK_BIAS

        q_i16 = np.searchsorted(uniq, qf).astype(np.int16)
        d_i16 = np.searchsorted(uniq, df).astype(np.int16)

        # one 512-idx gather per (chunk, batch): block (h, beta) covers
        # batch 4h+beta's 512 doc tokens; pre-arranged [128, h*b*32]
        d_idx = (
            np.stack(
                [
                    np.stack(
                        [
                            _wrap16(
                                d_i16[(4 * h + beta) * 512 : (4 * h + beta + 1) * 512],
                                512 // 16,
                            )
                            for beta in range(4)
                        ]
                    )
                    for h in range(DCHUNKS)
                ]
            )  # [h, b, 128, 32]
            .transpose(2, 0, 1, 3)
            .reshape(128, -1)
        )
        d_idx = np.ascontiguousarray(d_idx)
        q_idx = np.stack(
            [_wrap16(q_i16[g * 512 : (g + 1) * 512], 512 // 16) for g in range(2)]
        )

        qtokf = qf.reshape(DCHUNKS, 128).T.astype(np.float32)  # [128, 8]
        qm = (qf > 0).astype(np.float32) * 0.01
        qm001_a = qm.reshape(DCHUNKS, 128).T.astype(np.float32)
        qm88_a = np.repeat(qm001_a, NK, axis=1)  # [128, 88]
        d_tokf = np.ascontiguousarray(
            dt_c.reshape(DCHUNKS, 4, 512).transpose(1, 0, 2).reshape(4, -1)
        ).astype(np.float32)

        in_maps.append(
            {
                "mtab": mtab,
                "d_idx": d_idx,
                "q_idx": q_idx,
                "s_sel": s_sel,
                "s_selT": np.ascontiguousarray(s_sel.T),
                "d_tokf": d_tokf,
                "q_tokf": qtokf,
                "qm88": qm88_a,
                "w88": np.tile(
                    np.asarray(dense_w, dtype=np.float32).reshape(1, NK),
                    (4, DCHUNKS),
                ),
                "beff": (
                    np.asarray(dense_b, np.float32).reshape(-1)[0]
                    + np.asarray(dense_w, np.float32).reshape(-1)[0]
                    * 0.01
                    * LNC
                    * (qt_c > 0).sum(axis=1).reshape(DCHUNKS, 4).T
                ).astype(np.float32),
                "derfb": derfb,
                "qones": np.ones((1, NQTOK), dtype=ml_dtypes.bfloat16),
            }
        )
    return in_maps


def _install_loud_hook():
    # surface exceptions raised inside the PJRT compile callback, which are
    # otherwise swallowed by the C++ layer
    import traceback
    from concourse import bass2jax

    if getattr(bass2jax, "_loud_hook_installed", False):
        return
    orig = bass2jax.neuronx_cc_hook

    def loud(*a, **k):
        try:
            return orig(*a, **k)
        except BaseException:
            traceback.print_exc()
            raise

    bass2jax.neuronx_cc_hook = loud
    bass2jax._loud_hook_installed = True


LAST_RESULTS = None


def kernel(query_tokens, doc_tokens, embed_table, dense_w, dense_b):
    global LAST_RESULTS
    _install_loud_hook()
    from concourse.bass_utils import run_bass_kernel_spmd

    nc = _build_program()
    in_maps = _host_prep(query_tokens, doc_tokens, embed_table, dense_w, dense_b)
    res = run_bass_kernel_spmd(nc, in_maps, list(range(NCORES)))
    LAST_RESULTS = res
    out = np.empty((B,), dtype=np.float32)
    for c in range(NCORES):
        arr = res.results[c]["out"]  # [4, 8]: batch 4h+beta at [beta, h]
        out[c * BLOC : (c + 1) * BLOC] = arr.T.reshape(BLOC)
    return out


# revision 62
# speedup vs baseline: 2.7351x; 1.0605x over previous
"""KNRM kernel for 8 Trainium2 NeuronCores (data-parallel over batch).

Per core (32 batches):
  - host: dedup this core's tokens (~16k unique < int16 max), build a
    pre-normalized bf16 mini-table [17472, 384] (300 emb dims + mask-bias
    column at 300: -1e6 for vocab id 0, else 0), remap token tensors to
    int16 mini-table indices replicated across 16-partition groups (the
    Q7 dma_gather ucode reads a copy per 16-partition channel group).
  - device: per 2048-token chunk, ONE dma_gather(transpose=True) delivers
    embeddings directly in [e, token] layout (partition p, free slot j
    holds element 128j+p), so the cosine matmuls need no PE transposes and
    no PSUM->SBUF copies. Masking is folded into the contraction via the
    bias column (query side forced to 1.0). Gaussian kernel pooling runs
    as ONE scalar-engine pass per kernel using Derivative_Erf
    (d/dx erf = 2/sqrt(pi) * exp(-x^2)) with free-dim accumulation,
    reading cos straight from PSUM; the 2/sqrt(pi) factor is undone by
    the Ln(scale=sqrt(pi)/2) in the tail. k0 (sigma=1e-4, exact token
    match) is a DVE token-equality count scaled by 2/sqrt(pi) to share
    the same tail.
"""

import sys

sys.path.insert(0, "/opt/trn_rl_repo")

import numpy as np

B, Q, D, V, E = 256, 20, 512, 100000, 300
NCORES = 8
BLOC = B // NCORES  # 32 batches per core
QPAD = 32  # query slots per batch (20 real + 12 pad)
NQTOK = BLOC * QPAD  # 1024 query gather slots per core
DCHUNKS = 8  # doc chunks per core
DCTOK = 2048  # doc tokens per chunk (= 4 batches)
NK = 11
MSLOT = 384  # mini-table row elems (bf16) -> 768B, 256B-multiple
MROWS = 17472  # >= max unique tokens per core (16384 doc + 1024 q)
BIAS_COL = 300
MASK_BIAS = -1.0e6

SQRT50 = float(np.sqrt(50.0))
LN_SCALE = float(np.sqrt(np.pi) / 2.0)  # undo derf's 2/sqrt(pi)
K0_SCALE = float(2.0 / np.sqrt(np.pi))
CLIP = 1e-10 / LN_SCALE
# Ln(LN_SCALE*x) applied to the raw k0 count leaves an extra ln(LN_SCALE);
# the correction (-ln(LN_SCALE) per valid q row) is folded into the dense
# bias on the host
LNC = float(-np.log(LN_SCALE))


def _mus(n):
    l = [1.0]
    bs = 2.0 / (n - 1)
    l.append(1 - bs / 2)
    for i in range(1, n - 1):
        l.append(l[i] - bs)
    return l


MUS = _mus(NK)

_prog_cache = {}
DEBUG = False


def _build_program():
    key = ("nc", DEBUG)
    if key in _prog_cache:
        return _prog_cache[key]

    import concourse.bass as bass
    import concourse.bacc as bacc
    import concourse.mybir as mybir
    import concourse.tile as tile
    from concourse import library_config

    f32 = mybir.dt.float32
    bf16 = mybir.dt.bfloat16
    i16 = mybir.dt.int16
    AF = mybir.ActivationFunctionType
    ALU = mybir.AluOpType

    nc = bacc.Bacc(
        "TRN2",
        target_bir_lowering=False,
        debug=False,
        num_devices=NCORES,
        num_swdge_queues=4,
    )

    mtab = nc.dram_tensor("mtab", [MROWS, MSLOT], bf16, kind="ExternalInput").ap()
    d_idx = nc.dram_tensor(
        "d_idx", [128, DCHUNKS * 4 * (512 // 16)], i16, kind="ExternalInput"
    ).ap()
    q_idx = nc.dram_tensor(
        "q_idx", [2, 128, 512 // 16], i16, kind="ExternalInput"
    ).ap()
    s_sel = nc.dram_tensor("s_sel", [128, 4], f32, kind="ExternalInput").ap()
    d_tokf = nc.dram_tensor(
        "d_tokf", [128, DCHUNKS * 512], f32, kind="ExternalInput"
    ).ap()
    q_tokf = nc.dram_tensor("q_tokf", [128, DCHUNKS], f32, kind="ExternalInput").ap()
    qm88 = nc.dram_tensor(
        "qm88", [128, DCHUNKS * NK], f32, kind="ExternalInput"
    ).ap()
    w88 = nc.dram_tensor("w88", [4, DCHUNKS * NK], f32, kind="ExternalInput").ap()
    beff = nc.dram_tensor("beff", [4, DCHUNKS], f32, kind="ExternalInput").ap()

    derfb = nc.dram_tensor("derfb", [128, NK], f32, kind="ExternalInput").ap()
    qones = nc.dram_tensor("qones", [1, NQTOK], bf16, kind="ExternalInput").ap()
    out = nc.dram_tensor("out", [4, DCHUNKS], f32, kind="ExternalOutput").ap()
    dbg_pkq = (
        nc.dram_tensor("dbg_pkq", [DCHUNKS, 128, NK], f32, kind="ExternalOutput").ap()
        if DEBUG
        else None
    )
    dbg_cos = (
        nc.dram_tensor("dbg_cos", [DCHUNKS, 128, 512], f32, kind="ExternalOutput").ap()
        if DEBUG
        else None
    )

    with tile.TileContext(nc) as tc:
        import contextlib

        with contextlib.ExitStack() as ctx:
            const_pool = ctx.enter_context(tc.tile_pool(name="consts", bufs=1))
            qp = ctx.enter_context(tc.tile_pool(name="qprep", bufs=1))
            dpool = ctx.enter_context(tc.tile_pool(name="demb", bufs=3))
            pkpool = ctx.enter_context(tc.tile_pool(name="pk", bufs=1))
            scr = ctx.enter_context(tc.tile_pool(name="scr", bufs=2))
            psum = ctx.enter_context(
                tc.tile_pool(name="psum", bufs=2, space="PSUM")
            )
            psum1 = ctx.enter_context(
                tc.tile_pool(name="psum1", bufs=1, space="PSUM")
            )

            nc.gpsimd.load_library(library_config.mlp)

            s_sel_t = const_pool.tile([128, 4], f32)
            nc.sync.dma_start(out=s_sel_t[:], in_=s_sel[:])
            w88_t = const_pool.tile([4, DCHUNKS * NK], f32)
            nc.sync.dma_start(out=w88_t[:], in_=w88[:])
            beff_t = const_pool.tile([4, DCHUNKS], f32)
            nc.sync.dma_start(out=beff_t[:], in_=beff[:])
            derfb_t = const_pool.tile([128, NK], f32)
            nc.sync.dma_start(out=derfb_t[:], in_=derfb[:])
            qtokf_t = const_pool.tile([128, DCHUNKS], f32)
            nc.sync.dma_start(out=qtokf_t[:], in_=q_tokf[:])
            qm88_t = const_pool.tile([128, DCHUNKS * NK], f32)
            nc.sync.dma_start(out=qm88_t[:], in_=qm88[:])

            # ---------------- Q gather (transposed), 2x512 idxs ----------------
            # queue_num must track the global SWDGE issue index mod 4 so that
            # Tile's round-robin DMA-sem lanes (mod 8) never mix queues: a
            # lane shared by two queues completes out of issue order and the
            # lane's wait threshold passes early (observed as stale dT reads)
            gather_i = 0
            qT = qp.tile([128, 2 * 3 * 512], bf16)
            qT4 = qT[:].rearrange("p (g j n) -> p g j n", g=2, j=3)
            for g in range(2):
                qi = qp.tile([128, 512 // 16], i16, tag=f"qi{g}", name=f"qi{g}")
                nc.sync.dma_start(out=qi[:], in_=q_idx[g])
                nc.gpsimd.dma_gather(
                    out_ap=qT4[:, g],
                    in_ap=mtab[:],
                    idxs_ap=qi[:],
                    num_idxs=512,
                    num_idxs_reg=512,
                    elem_size=MSLOT,
                    transpose=True,
                    queue_num=gather_i % 4,
                )
                gather_i += 1
                # query-side bias multiplier: force e-row 300 (tile 2, part 44)
                nc.sync.dma_start(
                    out=qT4[44:45, g : g + 1, 2, :], in_=qones[:, 512 * g : 512 * (g + 1)]
                )

            # all doc-gather indices and token floats land in two upfront DMAs
            # (host pre-arranged to these layouts)
            di_all = qp.tile([128, DCHUNKS * 4 * (512 // 16)], i16)
            di4 = di_all[:].rearrange("p (h b n) -> p h b n", h=DCHUNKS, b=4)
            nc.sync.dma_start(out=di_all[:], in_=d_idx[:])
            # doc tokens pre-broadcast to all 128 partitions on the host, so
            # the k0 equality check needs no PE broadcast matmul
            dtf_all = qp.tile([128, DCHUNKS * 512], f32)
            dtf3 = dtf_all[:].rearrange("p (h n) -> p h n", h=DCHUNKS)
            nc.sync.dma_start(out=dtf_all[:], in_=d_tokf[:])

            # ---------------- main loop over chunk pairs ----------------
            # all chunks' pooled sums live in one [128, 8*11] tile: chunk h
            # owns columns 11h..11h+11 (k0 at 11h)
            pkq_all = pkpool.tile([128, DCHUNKS * NK], f32, tag="pkqall")
            pk3 = pkq_all[:].rearrange("p (h k) -> p h k", k=NK)
            groups = [(0, 1, 2), (3, 4, 5), (6, 7)]
            for grp in groups:
                glen = len(grp)
                # the group's chunks share one [128, 512*glen] PSUM cos tile
                # so each derf pass covers the whole group; the per-chunk sums
                # come from a segmented DVE reduce afterwards
                cosfull = psum.tile([128, 512 * 3], f32, tag="cos")
                cos = cosfull[:, : 512 * glen]
                for j, h in enumerate(grp):
                    dT = dpool.tile([128, 4 * 3 * 512], bf16, tag="demb")
                    dT4 = dT[:].rearrange("p (b j n) -> p b j n", b=4, j=3)
                    for beta in range(4):
                        nc.gpsimd.dma_gather(
                            out_ap=dT4[:, beta],
                            in_ap=mtab[:],
                            idxs_ap=di4[:, h, beta],
                            num_idxs=512,
                            num_idxs_reg=512,
                            elem_size=MSLOT,
                            transpose=True,
                            queue_num=gather_i % 4,
                        )
                        gather_i += 1



                    for beta in range(4):
                        b_glob = 4 * h + beta
                        g, qs = b_glob // 16, QPAD * (b_glob % 16)
                        cob = cos[32 * beta : 32 * beta + 32, 512 * j : 512 * (j + 1)]
                        nc.tensor.matmul(
                            out=cob,
                            lhsT=qT4[:, g, 0, qs : qs + QPAD],
                            rhs=dT4[:, beta, 0, :],
                            start=True,
                            stop=False,
                            tile_position=(0, 32 * beta),
                        )
                        nc.tensor.matmul(
                            out=cob,
                            lhsT=qT4[:, g, 1, qs : qs + QPAD],
                            rhs=dT4[:, beta, 1, :],
                            start=False,
                            stop=False,
                            tile_position=(0, 32 * beta),
                        )
                        nc.tensor.matmul(
                            out=cob,
                            lhsT=qT4[0:45, g, 2, qs : qs + QPAD],
                            rhs=dT4[0:45, beta, 2, :],
                            start=False,
                            stop=True,
                            tile_position=(0, 32 * beta),
                        )

                    # k0: exact-token-match count
                    cmp = scr.tile([128, 512], f32, tag="cmp")
                    nc.vector.tensor_scalar(
                        out=cmp[:],
                        in0=dtf3[:, h],
                        scalar1=qtokf_t[:, h : h + 1],
                        scalar2=0.0,
                        op0=ALU.is_equal,
                        op1=ALU.add,
                        accum_out=pk3[:, h, 0:1],
                    )

                    if DEBUG:
                        cos_sb = scr.tile([128, 512], f32, tag="cossb", name="cos_sb")
                        nc.vector.tensor_copy(
                            out=cos_sb[:], in_=cos[:, 512 * j : 512 * (j + 1)]
                        )
                        nc.sync.dma_start(out=dbg_cos[h], in_=cos_sb[:])

                # Gaussian kernels 1..10: one derf pass over the whole group,
                # then a segmented reduce for the per-chunk sums
                for k in range(1, NK):
                    sim = scr.tile([128, 512 * glen], bf16, tag=f"sim{glen}")
                    nc.scalar.activation(
                        out=sim[:],
                        in_=cos[:],
                        func=AF.Derivative_Erf,
                        scale=SQRT50,
                        bias=derfb_t[:, k : k + 1],
                    )
                    nc.vector.tensor_reduce(
                        out=pk3[:, grp[0] : grp[0] + glen, k : k + 1],
                        in_=sim[:].rearrange("p (c n) -> p c n", c=glen),
                        axis=mybir.AxisListType.X,
                        op=ALU.add,
                    )

            if DEBUG:
                for h in range(DCHUNKS):
                    nc.sync.dma_start(
                        out=dbg_pkq[h], in_=pkq_all[:, NK * h : NK * (h + 1)]
                    )

            # ---------------- tail: clip, log, mask, reduce, dense ----------------
            # batched over all chunks: 2 clips, 2 Ln passes (k0 is a raw count,
            # no 2/sqrt(pi) factor -> own clip+scale), 1 mask-mult, 1 matmul
            pk3 = pkq_all[:].rearrange("p (h k) -> p h k", k=NK)
            nc.vector.tensor_scalar(
                out=pk3[:, :, 0:1], in0=pk3[:, :, 0:1], scalar1=1e-10, scalar2=None,
                op0=ALU.max,
            )
            nc.vector.tensor_scalar(
                out=pk3[:, :, 1:NK], in0=pk3[:, :, 1:NK], scalar1=CLIP, scalar2=None,
                op0=ALU.max,
            )
            # one Ln for all 88 columns (forced after the last derf since it
            # reads the whole pkq tile -> exactly 2 ACT table loads per run);
            # the spurious ln(LN_SCALE) this adds to the k0 columns is undone
            # via the host-folded dense bias (beff)
            lnp = pkpool.tile([128, DCHUNKS * NK], f32, tag="lnpall")
            nc.scalar.activation(
                out=lnp[:], in_=pkq_all[:], func=AF.Ln, scale=LN_SCALE
            )
            nc.vector.tensor_tensor(
                out=lnp[:], in0=lnp[:], in1=qm88_t[:], op=ALU.mult
            )
            pkp = psum1.tile([4, DCHUNKS * NK], f32, tag="pkp")
            nc.tensor.matmul(
                out=pkp[:],
                lhsT=s_sel_t[:],
                rhs=lnp[:],
                start=True,
                stop=True,
            )
            pks = pkpool.tile([4, DCHUNKS * NK], f32, tag="pks")
            nc.vector.tensor_tensor(
                out=pks[:], in0=pkp[:], in1=w88_t[:], op=ALU.mult
            )
            out_acc = pkpool.tile([4, DCHUNKS], f32, tag="outacc")
            pks3 = pks[:].rearrange("p (h k) -> p h k", k=NK)
            for h in range(DCHUNKS):
                nc.vector.reduce_sum(
                    out=out_acc[:, h : h + 1],
                    in_=pks3[:, h],
                    axis=mybir.AxisListType.X,
                )
            nc.vector.tensor_tensor(
                out=out_acc[:], in0=out_acc[:], in1=beff_t[:], op=ALU.add
            )
            nc.sync.dma_start(out=out[:], in_=out_acc[:])

    nc.compile()
    _prog_cache[key] = nc
    return nc


def _wrap16(idx, ncols):
    """[N] int16 -> [128, ncols] with idx i at [i%16, i//16], replicated
    across all eight 16-partition groups (Q7 channel copies)."""
    a = np.asarray(idx, dtype=np.int16).reshape(ncols, 16).T  # [16, ncols]
    return np.tile(a, (8, 1))


def _host_prep(query_tokens, doc_tokens, embed_table, dense_w, dense_b):
    import ml_dtypes

    emb = np.ascontiguousarray(embed_table, dtype=np.float32)
    norms = np.sqrt(np.sum(emb.astype(np.float64) ** 2, axis=1))
    n_emb = emb / np.maximum(norms, 1e-13).astype(np.float32)[:, None]

    qt = np.asarray(query_tokens).astype(np.int64)
    dt = np.asarray(doc_tokens).astype(np.int64)

    s_sel = np.zeros((128, 4), dtype=np.float32)
    for p in range(128):
        s_sel[p, p // 32] = 1.0

    derfb = np.tile(
        (-SQRT50 * np.asarray(MUS, dtype=np.float32)).reshape(1, NK), (128, 1)
    )

    in_maps = []
    for c in range(NCORES):
        qt_c = qt[c * BLOC : (c + 1) * BLOC]  # [32, 20]
        dt_c = dt[c * BLOC : (c + 1) * BLOC]  # [32, 512]
        q_pad = np.zeros((BLOC, QPAD), dtype=np.int64)
        q_pad[:, :Q] = qt_c
        qf = q_pad.reshape(-1)  # [1024] slot order 32b+i
        df = dt_c.reshape(-1)  # [16384]

        uniq = np.unique(np.concatenate([qf, df]))
        assert len(uniq) <= MROWS, len(uniq)
        mtab = np.zeros((MROWS, MSLOT), dtype=ml_dtypes.bfloat16)
        mtab[: len(uniq), :E] = n_emb[uniq].astype(ml_dtypes.bfloat16)
        z = np.searchsorted(uniq, 0)
        if z < len(uniq) and uniq[z] == 0:
            mtab[z, :E] = 0
            mtab[z, BIAS_COL] = MASK_BIAS

        q_i16 = np.searchsorted(uniq, qf).astype(np.int16)
        d_i16 = np.searchsorted(uniq, df).astype(np.int16)

        # one 512-idx gather per (chunk, batch): block (h, beta) covers
        # batch 4h+beta's 512 doc tokens; pre-arranged [128, h*b*32]
        d_idx = (
            np.stack(
                [
                    np.stack(
                        [
                            _wrap16(
                                d_i16[(4 * h + beta) * 512 : (4 * h + beta + 1) * 512],
                                512 // 16,
                            )
                            for beta in range(4)
                        ]
                    )
                    for h in range(DCHUNKS)
                ]
            )  # [h, b, 128, 32]
            .transpose(2, 0, 1, 3)
            .reshape(128, -1)
        )
        d_idx = np.ascontiguousarray(d_idx)
        q_idx = np.stack(
            [_wrap16(q_i16[g * 512 : (g + 1) * 512], 512 // 16) for g in range(2)]
        )

        qtokf = qf.reshape(DCHUNKS, 128).T.astype(np.float32)  # [128, 8]
        qm = (qf > 0).astype(np.float32) * 0.01
        qm001_a = qm.reshape(DCHUNKS, 128).T.astype(np.float32)
        qm88_a = np.repeat(qm001_a, NK, axis=1)  # [128, 88]
        # doc tokens broadcast to [128, h*512]: partition p holds batch p//32
        d_tokf = np.ascontiguousarray(
            np.repeat(
                dt_c.reshape(DCHUNKS, 4, 512).transpose(1, 0, 2), 32, axis=0
            ).reshape(128, -1)
        ).astype(np.float32)

        in_maps.append(
            {
                "mtab": mtab,
                "d_idx": d_idx,
                "q_idx": q_idx,
                "s_sel": s_sel,
                "d_tokf": d_tokf,
                "q_tokf": qtokf,
                "qm88": qm88_a,
                "w88": np.tile(
                    np.asarray(dense_w, dtype=np.float32).reshape(1, NK),
                    (4, DCHUNKS),
                ),
                "beff": (
                    np.asarray(dense_b, np.float32).reshape(-1)[0]
                    + np.asarray(dense_w, np.float32).reshape(-1)[0]
                    * 0.01
                    * LNC
                    * (qt_c > 0).sum(axis=1).reshape(DCHUNKS, 4).T
                ).astype(np.float32),
                "derfb": derfb,
                "qones": np.ones((1, NQTOK), dtype=ml_dtypes.bfloat16),
            }
        )
    return in_maps


def _install_loud_hook():
    # surface exceptions raised inside the PJRT compile callback, which are
    # otherwise swallowed by the C++ layer
    import traceback
    from concourse import bass2jax

    if getattr(bass2jax, "_loud_hook_installed", False):
        return
    orig = bass2jax.neuronx_cc_hook

    def loud(*a, **k):
        try:
            return orig(*a, **k)
        except BaseException:
            traceback.print_exc()
            raise

    bass2jax.neuronx_cc_hook = loud
    bass2jax._loud_hook_installed = True


LAST_RESULTS = None


def kernel(query_tokens, doc_tokens, embed_table, dense_w, dense_b):
    global LAST_RESULTS
    _install_loud_hook()
    from concourse.bass_utils import run_bass_kernel_spmd

    nc = _build_program()
    in_maps = _host_prep(query_tokens, doc_tokens, embed_table, dense_w, dense_b)
    res = run_bass_kernel_spmd(nc, in_maps, list(range(NCORES)))
    LAST_RESULTS = res
    out = np.empty((B,), dtype=np.float32)
    for c in range(NCORES):
        arr = res.results[c]["out"]  # [4, 8]: batch 4h+beta at [beta, h]
        out[c * BLOC : (c + 1) * BLOC] = arr.T.reshape(BLOC)
    return out
